# revision 1
# baseline (speedup 1.0000x reference)
"""Trainium2 Bass kernel for nn_Agent_214748364878 (sparse_attention).

Strategy: pure data parallel over batch B=64 -> 8 batches per core.
The reference materializes huge [H,B,M,N,KS] glimpse tensors; instead we use
the algebraic identity  Q . (Kstat + ndf @ Wk)  =  Q . Kstat + (Q @ Wk) . ndf
so every big tensor is streamed exactly once through small block-diagonal
matmuls on the PE.

Shapes: B=64, M=5 vehicles, N=1000 nodes, D=128, H=8 heads, KS=16.
Output: softmax probs [64, 5000] (joint softmax over M*N per batch).
"""

import math
import numpy as np

B, M, N, D, H = 64, 5, 1000, 128, 8
KS = D // H
NCORES = 8
BL = B // NCORES        # 8 batches per core
NPAD = 1024             # N padded to 8 chunks of 128
NCHUNK = 8
MF = M * 8              # 40 = (m, feature) pairs
HM = H * M              # 40 = (head, vehicle) rows
MASKVAL = -1.0e6        # log(0) stand-in; exp() underflows to exactly 0.0

_CACHE = {}


def _build_program():
    import concourse.bass as bass
    import concourse.bacc as bacc
    import concourse.tile as tile
    from concourse import mybir

    f32 = mybir.dt.float32
    nc = bacc.Bacc("TRN2", target_bir_lowering=False, debug=False)

    # ---- DRAM tensors (per-core inputs, host-prearranged layouts) ----
    d_kst = nc.dram_tensor("kst", [BL, 128, NPAD], f32, kind="ExternalInput")
    d_lkst = nc.dram_tensor("lkst", [BL, 128, N], f32, kind="ExternalInput")
    d_vst = nc.dram_tensor("vst", [BL, NCHUNK, 128, 128], f32, kind="ExternalInput")
    d_ndft = nc.dram_tensor("ndft", [BL, MF + M, NPAD], f32, kind="ExternalInput")
    d_ndfn = nc.dram_tensor("ndfn", [BL, NCHUNK, 128, MF + 1], f32, kind="ExternalInput")
    d_prevT = nc.dram_tensor("prevT", [BL, 128, M], f32, kind="ExternalInput")
    d_vehT = nc.dram_tensor("vehT", [BL, 3, M], f32, kind="ExternalInput")
    d_fc = nc.dram_tensor("fc", [BL, 128, 1], f32, kind="ExternalInput")
    # constants (same on all cores)
    d_wpcvA = nc.dram_tensor("wpcvA", [128, 128], f32, kind="ExternalInput")
    d_wpcvB = nc.dram_tensor("wpcvB", [3, 128], f32, kind="ExternalInput")
    d_wk8 = nc.dram_tensor("wk8", [128, 8], f32, kind="ExternalInput")
    d_wl8 = nc.dram_tensor("wl8", [128, 8], f32, kind="ExternalInput")
    d_wvstk = nc.dram_tensor("wvstk", [MF, 128], f32, kind="ExternalInput")
    d_r8 = nc.dram_tensor("r8", [8, MF], f32, kind="ExternalInput")
    d_maskM = nc.dram_tensor("maskM", [MF, HM], f32, kind="ExternalInput")
    d_mask5 = nc.dram_tensor("mask5", [M, HM], f32, kind="ExternalInput")
    d_selT = nc.dram_tensor("selT", [HM, 128], f32, kind="ExternalInput")
    d_poT = nc.dram_tensor("poT", [128, 128], f32, kind="ExternalInput")
    d_ident = nc.dram_tensor("ident", [128, 128], f32, kind="ExternalInput")
    d_ones5 = nc.dram_tensor("ones5", [M, 1], f32, kind="ExternalInput")
    d_blkmask = nc.dram_tensor("blkmask", [128, HM], f32, kind="ExternalInput")
    d_ones15 = nc.dram_tensor("ones15", [1, M], f32, kind="ExternalInput")

    d_out = nc.dram_tensor("out", [BL, M, N], f32, kind="ExternalOutput")

    add = mybir.AluOpType.add
    mult = mybir.AluOpType.mult
    EXP = mybir.ActivationFunctionType.Exp
    TANH = mybir.ActivationFunctionType.Tanh

    with tile.TileContext(nc) as tc:
        with (
            tc.tile_pool(name="consts", bufs=1) as consts,
            tc.tile_pool(name="persist", bufs=1) as persist,
            tc.tile_pool(name="big", bufs=2) as big,
            tc.tile_pool(name="mid", bufs=2) as mid,
            tc.tile_pool(name="small", bufs=2) as small,
            tc.tile_pool(name="ps_mm", bufs=2, space="PSUM") as ps_mm,
            tc.tile_pool(name="ps_u", bufs=1, space="PSUM") as ps_u,
            tc.tile_pool(name="ps_s", bufs=1, space="PSUM") as ps_s,
            tc.tile_pool(name="ps_sm", bufs=2, space="PSUM") as ps_sm,
        ):
            # ---- load constants once ----
            def cload(dram, shape, tag):
                t = consts.tile(shape, f32, tag=tag)
                nc.sync.dma_start(t[:], dram.ap())
                return t

            wpcvA = cload(d_wpcvA, [128, 128], "c_wpcvA")
            wpcvB = cload(d_wpcvB, [3, 128], "c_wpcvB")
            wk8 = cload(d_wk8, [128, 8], "c_wk8")
            wl8 = cload(d_wl8, [128, 8], "c_wl8")
            wvstk = cload(d_wvstk, [MF, 128], "c_wvstk")
            r8 = cload(d_r8, [8, MF], "c_r8")
            maskM = cload(d_maskM, [MF, HM], "c_maskM")
            selT = cload(d_selT, [HM, 128], "c_selT")
            poT = cload(d_poT, [128, 128], "c_poT")
            ident = cload(d_ident, [128, 128], "c_ident")
            ones5 = cload(d_ones5, [M, 1], "c_ones5")
            ones15 = cload(d_ones15, [1, M], "c_ones15")
            blkmask = cload(d_blkmask, [128, HM], "c_blkmask")

            # persistent block-diagonal operand tiles
            lhsT1 = persist.tile([128, HM], f32)       # block-diag Q (scaled)
            lhsT2 = persist.tile([MF + M, HM], f32)    # block-diag QWk + mask rows
            lhsT3 = persist.tile([MF + M, M], f32)     # block-diag FWl + ident rows
            nc.vector.memset(lhsT1[:], 0.0)
            nc.sync.dma_start(lhsT2[MF:MF + M, :], d_mask5.ap())
            nc.sync.dma_start(lhsT3[MF:MF + M, :], d_mask5.ap()[:, 0:M])

            HALVES = [(0, 512), (512, 488)]

            for b in range(BL):
                # ---- stream per-batch inputs ----
                kst = big.tile([128, NPAD], f32, tag="kst")
                nc.sync.dma_start(kst[:], d_kst.ap()[b])
                lkst = big.tile([128, N], f32, tag="lkst")
                nc.sync.dma_start(lkst[:], d_lkst.ap()[b])
                vst = big.tile([128, NCHUNK, 128], f32, tag="vst")
                nc.sync.dma_start(vst[:], d_vst.ap()[b].rearrange("c p k -> p c k"))
                ndft = mid.tile([MF + M, NPAD], f32, tag="ndft")
                nc.sync.dma_start(ndft[:], d_ndft.ap()[b])
                ndfn = mid.tile([128, NCHUNK, MF + 1], f32, tag="ndfn")
                nc.sync.dma_start(ndfn[:], d_ndfn.ap()[b].rearrange("c p k -> p c k"))
                prevT = small.tile([128, M], f32, tag="prevT")
                nc.sync.dma_start(prevT[:], d_prevT.ap()[b])
                vehT = small.tile([3, M], f32, tag="vehT")
                nc.sync.dma_start(vehT[:], d_vehT.ap()[b])
                fc = small.tile([128, 1], f32, tag="fc")
                nc.sync.dma_start(fc[:], d_fc.ap()[b])
                maskb5 = mid.tile([M, N], f32, tag="maskb5")
                nc.sync.dma_start(maskb5[:], d_ndft.ap()[b, MF:MF + M, 0:N])

                # ---- query: qT[d, m] = W_pcv @ cvs.T + fc ----
                ps_q = ps_sm.tile([128, M], f32, tag="sm")
                nc.tensor.matmul(ps_q[:], wpcvA[:], prevT[:], start=True, stop=False)
                nc.tensor.matmul(ps_q[:], wpcvB[:], vehT[:], start=False, stop=True)
                qT = small.tile([128, M], f32, tag="qT")
                nc.vector.tensor_scalar_add(qT[:], ps_q[:], fc[:])

                # scatter into block-diag lhsT1: broadcast qT along h, mask
                nc.vector.tensor_mul(
                    lhsT1[:].rearrange("p (h m) -> p h m", h=H),
                    qT[:, None, :].broadcast_to([128, H, M]),
                    blkmask[:].rearrange("p (h m) -> p h m", h=H),
                )

                # ---- QWk[h,m,f] via wk8.T @ lhsT1 -> [8, 40] ----
                ps_qwk = ps_sm.tile([8, HM], f32, tag="sm")
                nc.tensor.matmul(ps_qwk[:], wk8[:], lhsT1[:])
                qwk = small.tile([8, HM], f32, tag="qwks")
                nc.vector.tensor_copy(qwk[:], ps_qwk[:])
                # replicate across m' (r8.T @ qwk -> [40, 40]) then mask
                ps_rep = ps_sm.tile([MF, HM], f32, tag="sm")
                nc.tensor.matmul(ps_rep[:], r8[:], qwk[:])
                nc.vector.tensor_mul(lhsT2[0:MF, :], ps_rep[:], maskM[:])

                # ---- compatT per n-chunk + exp + PV + S (no transposes) ----
                ET = mid.tile([128, NCHUNK * HM], f32, tag="ET")
                ps_U = ps_u.tile([HM, 128], f32, tag="U")
                ps_S = ps_s.tile([HM, MF + 1], f32, tag="S")
                for c in range(NCHUNK):
                    ps_ct = ps_mm.tile([128, HM], f32, tag="ct")
                    nc.tensor.matmul(ps_ct[:], kst[:, c * 128:(c + 1) * 128],
                                     lhsT1[:], start=True, stop=False)
                    nc.tensor.matmul(ps_ct[:], ndft[:, c * 128:(c + 1) * 128],
                                     lhsT2[:], start=False, stop=True)
                    nc.scalar.activation(ET[:, c * HM:(c + 1) * HM], ps_ct[:],
                                         EXP, scale=0.25)
                    nc.tensor.matmul(ps_U[:], ET[:, c * HM:(c + 1) * HM],
                                     vst[:, c, :], start=(c == 0), stop=False)
                    nc.tensor.matmul(ps_S[:], ET[:, c * HM:(c + 1) * HM],
                                     ndfn[:, c, :], start=(c == 0),
                                     stop=(c == NCHUNK - 1))
                # row sums came along as ndfn's ones column -> S[:, 40]
                r40 = small.tile([HM, 1], f32, tag="r40")
                nc.vector.tensor_copy(r40[:], ps_S[:, MF:MF + 1])

                # Z per head broadcast to (h*16+k) partitions, then 1/Z
                ps_z = ps_sm.tile([128, 1], f32, tag="sm")
                nc.tensor.matmul(ps_z[:], selT[:], r40[:])
                zbc = small.tile([128, 1], f32, tag="zbc")
                nc.vector.tensor_copy(zbc[:], ps_z[:])
                zinv = small.tile([128, 1], f32, tag="zinv")
                nc.vector.reciprocal(zinv[:], zbc[:])

                # ---- U2 = masked(S.T) @ Wv_stack accumulated into U ----
                S_sb = small.tile([HM, MF], f32, tag="S_sb")
                nc.vector.tensor_copy(S_sb[:], ps_S[:, 0:MF])
                ps_ST = ps_sm.tile([MF, HM], f32, tag="sm")
                nc.tensor.transpose(ps_ST[:], S_sb[:], ident[:HM, :HM])
                SmT = small.tile([MF, HM], f32, tag="SmT")
                nc.vector.tensor_mul(SmT[:], ps_ST[:], maskM[:])
                nc.tensor.matmul(ps_U[:], SmT[:], wvstk[:], start=False, stop=True)

                # ---- heads -> concatT (normalize by 1/Z) ----
                U_sb = small.tile([HM, 128], f32, tag="U_sb")
                nc.vector.tensor_copy(U_sb[:], ps_U[:])
                ps_UT = ps_mm.tile([128, HM], f32, tag="ct")
                nc.tensor.transpose(ps_UT[:], U_sb[:], ident[:HM, :HM])
                utm = small.tile([128, HM], f32, tag="utm")
                nc.vector.tensor_mul(utm[:], ps_UT[:], blkmask[:])
                concU = small.tile([128, M], f32, tag="concU")
                nc.vector.tensor_reduce(
                    concU[:], utm[:].rearrange("p (h m) -> p m h", h=H),
                    axis=mybir.AxisListType.X, op=add)
                concT = small.tile([128, M], f32, tag="concT")
                nc.vector.tensor_scalar_mul(concT[:], concU[:], zinv[:])

                # ---- final_Q ----
                ps_fq = ps_sm.tile([128, M], f32, tag="sm")
                nc.tensor.matmul(ps_fq[:], poT[:], concT[:])
                fqT = small.tile([128, M], f32, tag="fqT")
                nc.vector.tensor_copy(fqT[:], ps_fq[:])

                # FWl -> block diag lhsT3
                ps_fwl = ps_sm.tile([8, M], f32, tag="sm")
                nc.tensor.matmul(ps_fwl[:], wl8[:], fqT[:])
                fwl = small.tile([8, M], f32, tag="fwls")
                nc.vector.tensor_copy(fwl[:], ps_fwl[:])
                ps_r3 = ps_sm.tile([MF, M], f32, tag="sm")
                nc.tensor.matmul(ps_r3[:], r8[:], fwl[:])
                nc.vector.tensor_mul(lhsT3[0:MF, :], ps_r3[:], maskM[:, 0:M])

                # ---- logits + tanh + mask + exp ----
                eL = mid.tile([M, N], f32, tag="eL")
                rL = small.tile([M, 2], f32, tag="rL")
                for i, (off, ln) in enumerate(HALVES):
                    ps_L = ps_mm.tile([M, 512], f32, tag="mm")
                    nc.tensor.matmul(ps_L[:, :ln], fqT[:], lkst[:, off:off + ln],
                                     start=True, stop=False)
                    nc.tensor.matmul(ps_L[:, :ln], lhsT3[:], ndft[:, off:off + ln],
                                     start=False, stop=True)
                    tl = small.tile([M, 512], f32, tag="tl")
                    nc.scalar.activation(tl[:, :ln], ps_L[:, :ln], TANH,
                                         scale=1.0 / math.sqrt(D))
                    pl = small.tile([M, 512], f32, tag="pl")
                    nc.vector.scalar_tensor_tensor(
                        pl[:, :ln], tl[:, :ln], 10.0,
                        maskb5[:, off:off + ln], op0=mult, op1=add)
                    nc.scalar.activation(eL[:, off:off + ln], pl[:, :ln], EXP,
                                         accum_out=rL[:, i:i + 1])
                rL5 = small.tile([M, 1], f32, tag="rL5")
                nc.vector.tensor_tensor(rL5[:], rL[:, 0:1], rL[:, 1:2], op=add)
                ps_z1 = ps_sm.tile([1, 1], f32, tag="sm")
                nc.tensor.matmul(ps_z1[:], ones5[:], rL5[:])
                z1 = small.tile([1, 1], f32, tag="z1s")
                nc.vector.tensor_copy(z1[:], ps_z1[:])
                zi1 = small.tile([1, 1], f32, tag="zi1")
                nc.vector.reciprocal(zi1[:], z1[:])
                ps_zb = ps_sm.tile([M, 1], f32, tag="sm")
                nc.tensor.matmul(ps_zb[:], ones15[:], zi1[:])
                zb5 = small.tile([M, 1], f32, tag="zb5")
                nc.vector.tensor_copy(zb5[:], ps_zb[:])

                outb = mid.tile([M, N], f32, tag="outb")
                nc.vector.tensor_scalar_mul(outb[:], eL[:], zb5[:])
                nc.sync.dma_start(d_out.ap()[b], outb[:])

    nc.compile()
    return nc


def _prep_inputs(inputs):
    """Host-side shard + relayout (numpy moves only, no arithmetic on data)."""
    gks = inputs["glimpse_K_static"]   # [H, B, 1, N, KS]
    gvs = inputs["glimpse_V_static"]
    lks = inputs["logit_K_static"]     # [B, 1, N, D]
    ndf = inputs["node_dynamic_features"]  # [B, M, N, 8]
    mask = inputs["feasibility_mask"]  # [B, M, N] bool
    prev = inputs["prev_node_embeddings"]  # [B, M, D]
    veh = inputs["vehicle_dynamic_features"]  # [B, M, 3]
    fc = inputs["fixed_context"]       # [B, 1, D]
    W_pcv = inputs["W_pcv"]            # [D, D+3]
    W_pns = inputs["W_pns"]            # [3D, 8]
    po = inputs["po_weight"]           # [D, D]

    f = np.float32
    # [B, 128, NPAD]: row h*16+k = Kstat[h, b, 0, :, k]; zero-padded n
    kst = np.zeros((B, 128, NPAD), dtype=f)
    kst[:, :, :N] = gks[:, :, 0].transpose(1, 0, 3, 2).reshape(B, 128, N)
    lkst = np.ascontiguousarray(lks[:, 0].transpose(0, 2, 1), dtype=f)  # [B,128,N]
    # [B, chunk, np, h*16+k]
    vpad = np.zeros((B, NPAD, 128), dtype=f)
    vpad[:, :N, :] = gvs[:, :, 0].transpose(1, 2, 0, 3).reshape(B, N, 128)
    vst = np.ascontiguousarray(vpad.reshape(B, NCHUNK, 128, 128))
    # ndft: [B, 45, NPAD] rows 0-39 = (m,f), rows 40-44 = mask bias per m
    # (padded n marked infeasible so exp() of padded compat is exactly 0)
    maskb = np.full((B, M, NPAD), MASKVAL, dtype=f)
    maskb[:, :, :N] = np.where(mask, np.float32(0.0), np.float32(MASKVAL))
    ndft = np.zeros((B, MF + M, NPAD), dtype=f)
    ndft[:, :MF, :N] = ndf.transpose(0, 1, 3, 2).reshape(B, MF, N)
    ndft[:, MF:, :] = maskb
    # ndfn: [B, chunk, np, (m,f)+ones] ; ones col counts only real n
    npad = np.zeros((B, NPAD, MF + 1), dtype=f)
    npad[:, :N, :MF] = ndf.transpose(0, 2, 1, 3).reshape(B, N, MF)
    npad[:, :N, MF] = 1.0
    ndfn = np.ascontiguousarray(npad.reshape(B, NCHUNK, 128, MF + 1))
    prevT = np.ascontiguousarray(prev.transpose(0, 2, 1), dtype=f)  # [B,128,M]
    vehT = np.ascontiguousarray(veh.transpose(0, 2, 1), dtype=f)    # [B,3,M]
    fcT = np.ascontiguousarray(fc.transpose(0, 2, 1), dtype=f)      # [B,128,1]

    # constants
    wpcvT = np.ascontiguousarray(W_pcv.T, dtype=f)          # [131, 128]
    wpcvA, wpcvB = wpcvT[:128], wpcvT[128:131]
    wk8 = np.ascontiguousarray(W_pns[128:256], dtype=f)     # [128, 8]
    wl8 = np.ascontiguousarray(W_pns[256:384], dtype=f)     # [128, 8]
    wvstk = np.ascontiguousarray(
        np.tile(W_pns[0:128].T.reshape(1, 8, 128), (M, 1, 1)).reshape(MF, 128),
        dtype=f)                                            # [(m,f), d]
    r8 = np.zeros((8, MF), dtype=f)
    for m in range(M):
        for ff in range(8):
            r8[ff, m * 8 + ff] = 1.0
    maskM = np.zeros((MF, HM), dtype=f)
    for m in range(M):
        for ff in range(8):
            for h in range(H):
                maskM[m * 8 + ff, h * M + m] = 1.0
    mask5 = np.zeros((M, HM), dtype=f)
    for m in range(M):
        for h in range(H):
            mask5[m, h * M + m] = 1.0
    selT = np.zeros((HM, 128), dtype=f)
    for h in range(H):
        for m in range(M):
            selT[h * M + m, h * KS:(h + 1) * KS] = 1.0
    poT = np.ascontiguousarray(po.T, dtype=f)
    ident = np.eye(128, dtype=f)
    ones5 = np.ones((M, 1), dtype=f)
    blkmask = np.zeros((128, HM), dtype=f)
    for h in range(H):
        blkmask[h * KS:(h + 1) * KS, h * M:(h + 1) * M] = 1.0
    ones15 = np.ones((1, M), dtype=f)

    consts = dict(wpcvA=wpcvA, wpcvB=wpcvB, wk8=wk8, wl8=wl8, wvstk=wvstk,
                  r8=r8, maskM=maskM, mask5=mask5, selT=selT, poT=poT,
                  ident=ident, ones5=ones5, ones15=ones15, blkmask=blkmask)

    in_maps = []
    for c in range(NCORES):
        sl = slice(c * BL, (c + 1) * BL)
        m = dict(kst=kst[sl], lkst=lkst[sl], vst=vst[sl], ndft=ndft[sl],
                 ndfn=ndfn[sl], prevT=prevT[sl], vehT=vehT[sl], fc=fcT[sl])
        m.update({k: v.copy() for k, v in consts.items()})
        in_maps.append(m)
    return in_maps


def kernel(**inputs) -> np.ndarray:
    from concourse import bass_utils

    if "nc" not in _CACHE:
        _CACHE["nc"] = _build_program()
    nc = _CACHE["nc"]
    in_maps = _prep_inputs(inputs)
    res = bass_utils.run_bass_kernel_spmd(nc, in_maps, core_ids=list(range(NCORES)))
    outs = [res.results[c]["out"].reshape(BL, M * N) for c in range(NCORES)]
    return np.concatenate(outs, axis=0).astype(np.float32)



# revision 3
# speedup vs baseline: 1.3710x; 1.3710x over previous
"""Trainium2 Bass kernel for nn_Agent_214748364878 (sparse_attention), v2.

Pure data parallel over batch B=64 -> 8 batches per core. Uses the algebraic
identity  Q . (Kstat + ndf @ Wk) = Q . Kstat + (QWk) . ndf  so the huge
[H,B,M,N,KS] tensors of the reference are never materialized; every big
tensor streams through the PE exactly once.

v2 vs v1: fp16 matmul operands everywhere (1 PE cycle/row instead of 4 for
fp32; validated max rel err 1.5e-3 vs the 2e-2 gate), one fused DMA blob per
batch instead of 7 transfers, n-on-partitions logits phase ([128,40] tiles
instead of [5,512]), prep/tail math batched across the core's 8 batches, and
full cross-batch pipelining (multi-buffered PSUM/SBUF pools, no persistent
in-place tiles on the critical path).

Shapes: B=64, M=5 vehicles, N=1000 nodes, D=128, H=8 heads, KS=16.
Output: softmax probs [64, 5000] (joint softmax over M*N per batch).
"""

import math
import numpy as np

B, M, N, D, H = 64, 5, 1000, 128, 8
KS = D // H
NCORES = 8
BL = B // NCORES          # 8 batches per core
NPAD = 1024
NCHUNK = 8
MF = M * 8                # 40 (m, feature) pairs
HM = H * M                # 40 (head, vehicle) pairs
BM = BL * M               # 40 (batch, vehicle) pairs
BHM = BL * HM             # 320
MASKVAL = -60000.0        # fits fp16; exp underflows to exactly 0

# blob free-dim column offsets (fp16 elements)
O_KST = 0                 # [128=(h,k), 1024=n]
O_LKST = 1024             # [128=d, 1024=n]
O_VST = 2048              # [128=n%128, (c, 128=(h,k))]
O_NDFN = 3072             # [128=n%128, (c, 41=(m,f)+ones)]
O_MASKT = 3400            # [128=n%128, (c, 5=m)]  logits mask, 0/-60000
O_MASK01 = 3440           # [128=n%128, (c, 5=m)]  attention mask, 0/1
MF1 = 41                  # ndfn cols per chunk incl ones
BLOBC = 3480

# cpack16 column offsets (fp16)
C_R8 = 0                  # [8, 40]
C_MASKMT = 40             # [40, 40]  (mf, hm) same-m
C_MASKB5 = 80             # [40, 5]   (mf, m') same-m
C_MASKMA = 85             # [40, 320] (mf, (b,h,m)) same-m
C_MASK5R = 405            # [5, 320]  (m', (b,h,m)) same-m
C_WVSTK = 725             # [40, 128] (mf, d) = Wv.T tiled
C_BLKM = 853              # [128, 40] ((h',k), (h,m)) same-h
C_POT = 893               # [128, 128] po.T
C_WL8 = 1021              # [128, 8] W_pns[256:384]
C_PO = 1029               # [128, 128] po (for on-device po.T @ Wl)
C_ONES = 1157             # [128, 1] ones
C_WK8 = 1158              # [128, 8] W_pns[128:256]
C16 = 1166

# cpack32 column offsets (fp32)
F_WPCVA = 0               # [128, 128] W_pcv[:, :128].T
F_WPCVB = 128             # [3, 128]   W_pcv[:, 128:].T
F_SELT = 256              # [40, 128]  ((h,m), (h',k)) same-h
F_ONES = 384              # [128, 1]   ones
F_ONER = 385              # [1, 128]   ones row
F_FCM = 513               # [128, 40]  fixed_context replicated per m
F_PREVT = 553             # [128, 40]  prev_node_embeddings.T
F_VEHT = 593              # [3, 40]    vehicle_dynamic_features.T
F_ONEM = 633              # [128, 128] all-ones (partition-sum broadcast)
F_POT = 761               # [128, 128] po.T (fp32 fq matmul)
F_PO = 889                # [128, 128] po (for on-device po.T @ Wl)
F_WL8 = 1017              # [128, 8]   W_pns[256:384] fp32
F32C = 1025

_CACHE = {}


def _build_program():
    import concourse.bass as bass
    import concourse.bacc as bacc
    import concourse.tile as tile
    from concourse import mybir

    f32 = mybir.dt.float32
    f16 = mybir.dt.float16
    nc = bacc.Bacc("TRN2", target_bir_lowering=False, debug=False)

    d_blob = nc.dram_tensor("blob", [BL, 128, BLOBC], f16, kind="ExternalInput")
    d_ndftp = nc.dram_tensor("ndftp", [BL // 2, 128, NPAD], f16,
                             kind="ExternalInput")
    d_cp16 = nc.dram_tensor("cp16", [128, C16], f16, kind="ExternalInput")
    d_cp32 = nc.dram_tensor("cp32", [128, F32C], f32, kind="ExternalInput")
    d_out = nc.dram_tensor("out", [128, BL * NCHUNK * M], f32,
                           kind="ExternalOutput")

    mult = mybir.AluOpType.mult
    add = mybir.AluOpType.add
    EXP = mybir.ActivationFunctionType.Exp
    TANH = mybir.ActivationFunctionType.Tanh

    with tile.TileContext(nc) as tc:
        with (
            tc.tile_pool(name="consts", bufs=1) as consts,
            tc.tile_pool(name="persist", bufs=1) as persist,
            tc.tile_pool(name="dmab", bufs=4) as dmab,
            tc.tile_pool(name="dman", bufs=3) as dman,
            tc.tile_pool(name="work", bufs=3) as work,
            tc.tile_pool(name="ps_ct", bufs=3, space="PSUM") as ps_ct_pool,
            tc.tile_pool(name="ps_ut", bufs=3, space="PSUM") as ps_ut_pool,
            tc.tile_pool(name="ps_tail", bufs=2, space="PSUM") as ps_tail_pool,
        ):
            cp16 = consts.tile([128, C16], f16)
            nc.scalar.dma_start(cp16[:], d_cp16.ap())
            cp32 = consts.tile([128, F32C], f32)
            nc.scalar.dma_start(cp32[:], d_cp32.ap())

            # ---------------- prep phase (once, all 8 batches) ----------------
            ps_q = ps_ct_pool.tile([128, BM], f32, tag="ct")
            nc.tensor.matmul(ps_q[:], cp32[:, F_WPCVA:F_WPCVA + 128],
                             cp32[:, F_PREVT:F_PREVT + BM],
                             start=True, stop=False)
            nc.tensor.matmul(ps_q[:], cp32[0:3, F_WPCVB:F_WPCVB + 128],
                             cp32[0:3, F_VEHT:F_VEHT + BM],
                             start=False, stop=True)
            qT_all = work.tile([128, BM], f32, tag="qT_all")
            nc.vector.tensor_tensor(qT_all[:], ps_q[:],
                                    cp32[:, F_FCM:F_FCM + BM], op=add)

            # block-diag Q, all batches: [128=(h,k), (b,h,m)]
            lhsT1 = persist.tile([128, BHM], f16)
            nc.vector.tensor_tensor(
                lhsT1[:].rearrange("p (b h m) -> p b h m", b=BL, h=H),
                qT_all[:].rearrange("p (b m) -> p b m", b=BL)[:, :, None, :]
                .broadcast_to([128, BL, H, M]),
                cp16[:, C_BLKM:C_BLKM + HM]
                .rearrange("p (h m) -> p h m", h=H)[:, None, :, :]
                .broadcast_to([128, BL, H, M]),
                op=mult)

            # per-head QWk replicated over m' -> lhsT2 rows 0:40; mask-bias
            # selector rows 40:45
            ps_qwk = ps_ct_pool.tile([8, BHM], f32, tag="ct")
            nc.tensor.matmul(ps_qwk[:], cp16[:, C_WK8:C_WK8 + 8], lhsT1[:])
            qwk_sb = work.tile([8, BHM], f16, tag="qwk_sb")
            nc.vector.tensor_copy(qwk_sb[:], ps_qwk[:])
            ps_rep = ps_ct_pool.tile([MF, BHM], f32, tag="ct")
            nc.tensor.matmul(ps_rep[:], cp16[0:8, C_R8:C_R8 + MF], qwk_sb[:])
            # duplicated at partition bases 0 and 64 to pair with the
            # two-batches-per-tile ndft layout (matmul requires equal
            # base partitions for lhsT and rhs)
            lhsT2 = persist.tile([64 + MF, BHM], f16)
            for nb in (0, 64):
                nc.vector.tensor_tensor(lhsT2[nb:nb + MF, :], ps_rep[:],
                                        cp16[0:MF, C_MASKMA:C_MASKMA + BHM],
                                        op=mult)

            # powl = po.T @ Wl (on-device weight fusion for the logits
            # dynamic path; lets FWl come straight from concT, parallel to fq)
            ps_powl = ps_tail_pool.tile([128, 8], f32, tag="tail")
            nc.tensor.matmul(ps_powl[:], cp16[:, C_PO:C_PO + 128],
                             cp16[:, C_WL8:C_WL8 + 8])
            powl = persist.tile([128, 8], f16)
            nc.vector.tensor_copy(powl[:], ps_powl[:])

            # ------------- per-batch pipeline, software-pipelined -------------
            out_all = persist.tile([128, BL * NCHUNK * M], f32)
            ndft_tiles = {}
            state = {}

            def stage_a(b):
                """DMA in + compat + exp + feasibility mask."""
                blob = dmab.tile([128, BLOBC], f16, tag="blob")
                # split by consumer stage: kst (compat) first, then
                # vst/ndfn/masks (acc + masks), then lkst (logits)
                nc.sync.dma_start(blob[:, 0:1024], d_blob.ap()[b][:, 0:1024])
                nc.sync.dma_start(blob[:, 2048:BLOBC],
                                  d_blob.ap()[b][:, 2048:BLOBC])
                nc.sync.dma_start(blob[:, 1024:2048],
                                  d_blob.ap()[b][:, 1024:2048])
                if b % 2 == 0:
                    nd = dman.tile([128, NPAD], f16, tag="ndft")
                    nc.scalar.dma_start(nd[:], d_ndftp.ap()[b // 2])
                    ndft_tiles[b // 2] = nd
                nb = 64 * (b % 2)
                ndft_t = ndft_tiles[b // 2]
                ps_ct = ps_ct_pool.tile([128, NCHUNK * HM], f32, tag="ct")
                for c in range(NCHUNK):
                    cs = slice(c * HM, (c + 1) * HM)
                    nc.tensor.matmul(ps_ct[:, cs],
                                     blob[:, O_KST + c * 128:O_KST + (c + 1) * 128],
                                     lhsT1[:, b * HM:(b + 1) * HM],
                                     start=True, stop=False)
                    nc.tensor.matmul(ps_ct[:, cs],
                                     ndft_t[nb:nb + MF, c * 128:(c + 1) * 128],
                                     lhsT2[nb:nb + MF, b * HM:(b + 1) * HM],
                                     start=False, stop=True)
                ETu = work.tile([128, NCHUNK * HM], f16, tag="ETu")
                nc.scalar.activation(ETu[:], ps_ct[:], EXP, scale=0.25)
                # 0/1 feasibility mask post-exp; Z/U/S all consume the masked
                # E so this is exact
                ET = work.tile([128, NCHUNK * HM], f16, tag="ET")
                nc.vector.tensor_tensor(
                    ET[:].rearrange("p (c h m) -> p c h m", c=NCHUNK, h=H),
                    ETu[:].rearrange("p (c h m) -> p c h m", c=NCHUNK, h=H),
                    blob[:, O_MASK01:O_MASK01 + NCHUNK * M]
                    .rearrange("p (c m) -> p c m", c=NCHUNK)[:, :, None, :]
                    .broadcast_to([128, NCHUNK, H, M]),
                    op=mult)
                state[b] = dict(blob=blob, ndft=ndft_t, nb=nb, ET=ET)

            def stage_b(p):
                """S/U/Z accumulation + attention tail for batch pair
                (2p, 2p+1) — pairing halves the per-batch chain length."""
                b0, b1 = 2 * p, 2 * p + 1
                st0, st1 = state[b0], state[b1]
                # ps_st: per-batch S^T (cols 0:40 / 41:81) + Z (cols 40 / 81)
                # ps_ut: per-batch U^T side by side [128, 80].
                # One spanning accumulation group per bank (first matmul
                # start=True covers the zero region; last has stop=True).
                tailt = ps_tail_pool.tile([128, 196], f32, tag="tail")
                ps_st = tailt[0:MF, 114:196]
                ps_ut = ps_ut_pool.tile([128, 2 * HM], f32, tag="ut")
                for i, st in ((0, st0), (1, st1)):
                    blob, ET = st["blob"], st["ET"]
                    so, uo = 41 * i, HM * i
                    for c in range(NCHUNK):
                        cs = slice(c * HM, (c + 1) * HM)
                        nc.tensor.matmul(ps_st[:, so:so + 40],
                                         blob[:, O_NDFN + c * MF1:O_NDFN + c * MF1 + MF],
                                         ET[:, cs],
                                         start=(c == 0 and i == 0), stop=False,
                                         skip_group_check=True)
                        nc.tensor.matmul(ps_st[:, so + 40:so + 41], ET[:, cs],
                                         cp16[:, C_ONES:C_ONES + 1],
                                         start=False, stop=False,
                                         skip_group_check=True)
                        nc.tensor.matmul(ps_ut[:, uo:uo + HM],
                                         blob[:, O_VST + c * 128:O_VST + (c + 1) * 128],
                                         ET[:, cs],
                                         start=(c == 0 and i == 0), stop=False,
                                         skip_group_check=True)
                # U2^T for both batches via masked S^T
                SmT = work.tile([MF, 2 * HM], f16, tag="SmT")
                nc.vector.tensor_tensor(
                    SmT[:].rearrange("p (i x) -> p i x", i=2),
                    ps_st[:].rearrange("p (i x) -> p i x", i=2)[:, :, 0:40],
                    cp16[0:MF, C_MASKMT:C_MASKMT + HM][:, None, :]
                    .broadcast_to([MF, 2, HM]),
                    op=mult)
                nc.tensor.matmul(ps_ut[:, 0:HM],
                                 cp16[0:MF, C_WVSTK:C_WVSTK + 128],
                                 SmT[:, 0:HM], start=False, stop=False,
                                 skip_group_check=True)
                nc.tensor.matmul(ps_ut[:, HM:2 * HM],
                                 cp16[0:MF, C_WVSTK:C_WVSTK + 128],
                                 SmT[:, HM:2 * HM], start=False, stop=True,
                                 skip_group_check=True)
                # per-head 1/Z for both batches in one matmul/reciprocal
                r40 = work.tile([MF, 2], f32, tag="r40")
                nc.vector.tensor_copy(
                    r40[:], ps_st[:].rearrange("p (i x) -> p i x", i=2)[:, :, 40])
                nc.tensor.matmul(tailt[:, 90:92],
                                 cp32[0:HM, F_SELT:F_SELT + 128], r40[:],
                                 skip_group_check=True)
                zinv = work.tile([128, 2], f32, tag="zinv")
                nc.vector.reciprocal(zinv[:], tailt[:, 90:92])
                # masked normalized U^T -> fq, FWl (accumulated over heads)
                utm = work.tile([128, 2 * HM], f16, tag="utm")
                for i in (0, 1):
                    nc.vector.scalar_tensor_tensor(
                        utm[:, HM * i:HM * (i + 1)],
                        ps_ut[:, HM * i:HM * (i + 1)], zinv[:, i:i + 1],
                        cp16[:, C_BLKM:C_BLKM + HM], op0=mult, op1=mult)
                for i in (0, 1):
                    for h in range(H):
                        hs = slice(HM * i + h * M, HM * i + (h + 1) * M)
                        nc.tensor.matmul(tailt[:, 80 + 5 * i:85 + 5 * i],
                                         cp16[:, C_POT:C_POT + 128], utm[:, hs],
                                         start=(h == 0 and i == 0), stop=False,
                                         skip_group_check=True)
                        nc.tensor.matmul(tailt[0:8, 94 + 5 * i:99 + 5 * i],
                                         powl[:], utm[:, hs],
                                         start=False,
                                         stop=(h == H - 1 and i == 1),
                                         skip_group_check=True)
                fqT = work.tile([128, 2 * M], f16, tag="fqT")
                nc.scalar.activation(fqT[:], tailt[:, 80:90],
                                     mybir.ActivationFunctionType.Copy)
                fwl = work.tile([8, 2 * M], f16, tag="fwl")
                nc.vector.tensor_copy(fwl[:], tailt[0:8, 94:104])
                nc.tensor.matmul(tailt[0:MF, 104:114],
                                 cp16[0:8, C_R8:C_R8 + MF], fwl[:],
                                 skip_group_check=True)
                lhsT3 = work.tile([64 + MF, 2 * M], f16, tag="lhsT3")
                for i in (0, 1):
                    nb = 64 * i
                    nc.vector.tensor_tensor(
                        lhsT3[nb:nb + MF, 5 * i:5 * (i + 1)],
                        tailt[0:MF, 104 + 5 * i:109 + 5 * i],
                        cp16[0:MF, C_MASKB5:C_MASKB5 + M],
                        op=mult)
                st0["tailt"] = st1["tailt"] = tailt
                st0["fqT"] = st1["fqT"] = fqT
                st0["lhsT3"] = st1["lhsT3"] = lhsT3

            def stage_c(p):
                """Logits + joint softmax + output for batch pair."""
                b0, b1 = 2 * p, 2 * p + 1
                st0, st1 = state.pop(b0), state.pop(b1)
                tailt = st0["tailt"]
                fqT, lhsT3 = st0["fqT"], st0["lhsT3"]
                for i, st in ((0, st0), (1, st1)):
                    blob, ndft_t, nb = st["blob"], st["ndft"], st["nb"]
                    for c in range(NCHUNK):
                        cs = slice(40 * i + c * M, 40 * i + (c + 1) * M)
                        nc.tensor.matmul(tailt[:, cs],
                                         blob[:, O_LKST + c * 128:O_LKST + (c + 1) * 128],
                                         fqT[:, 5 * i:5 * (i + 1)],
                                         start=True, stop=False,
                                         skip_group_check=True)
                        nc.tensor.matmul(tailt[:, cs],
                                         ndft_t[nb:nb + MF, c * 128:(c + 1) * 128],
                                         lhsT3[nb:nb + MF, 5 * i:5 * (i + 1)],
                                         start=False, stop=True,
                                         skip_group_check=True)
                tl = work.tile([128, 2 * NCHUNK * M], f32, tag="tl")
                nc.scalar.activation(tl[:], tailt[:, 0:80], TANH,
                                     scale=1.0 / math.sqrt(D))
                pl = work.tile([128, 2 * NCHUNK * M], f32, tag="pl")
                for i, st in ((0, st0), (1, st1)):
                    nc.vector.scalar_tensor_tensor(
                        pl[:, 40 * i:40 * (i + 1)], tl[:, 40 * i:40 * (i + 1)],
                        10.0, st["blob"][:, O_MASKT:O_MASKT + 40],
                        op0=mult, op1=add)
                eL = work.tile([128, 2 * NCHUNK * M], f32, tag="eL")
                rL = work.tile([128, 2], f32, tag="rL")
                nc.scalar.activation(eL[:, 0:40], pl[:, 0:40], EXP,
                                     accum_out=rL[:, 0:1])
                nc.scalar.activation(eL[:, 40:80], pl[:, 40:80], EXP,
                                     accum_out=rL[:, 1:2])
                nc.tensor.matmul(tailt[:, 92:94],
                                 cp32[:, F_ONEM:F_ONEM + 128], rL[:],
                                 skip_group_check=True)
                zbinv = work.tile([128, 2], f32, tag="zbinv")
                nc.vector.reciprocal(zbinv[:], tailt[:, 92:94])
                for i, b in ((0, b0), (1, b1)):
                    nc.vector.tensor_scalar_mul(
                        out_all[:, b * NCHUNK * M:(b + 1) * NCHUNK * M],
                        eL[:, 40 * i:40 * (i + 1)], zbinv[:, i:i + 1])

            import os
            STAGES = int(os.environ.get("KV2_STAGES", "3"))
            LAG = int(os.environ.get("KV2_LAG", "1"))
            ORDER = os.environ.get("KV2_ORDER", "abc")
            _stage_map = _CACHE.setdefault("stage_map", [])

            def _mark(tag, fn, *a):
                i0 = nc.next_id()
                fn(*a)
                _stage_map.append((tag, i0, nc.next_id()))

            for t in range(BL + 4):
                for s in ORDER:
                    if s == "a" and t < BL:
                        _mark(f"A{t}", stage_a, t)
                    if (s == "b" and STAGES >= 2 and t % 2 == 1
                            and 0 <= (t - 1) // 2 < BL // 2):
                        _mark(f"B{(t - 1) // 2}", stage_b, (t - 1) // 2)
                    if (s == "c" and STAGES >= 3 and t % 2 == 0
                            and 0 <= (t - 4) // 2 < BL // 2):
                        _mark(f"C{(t - 4) // 2}", stage_c, (t - 4) // 2)
            if STAGES >= 3:
                nc.sync.dma_start(d_out.ap(), out_all[:])
            else:
                nc.vector.memset(out_all[:], 0.0)
                nc.sync.dma_start(d_out.ap(), out_all[:])

    nc.compile()
    return nc


def _prep_inputs(inputs):
    """Host-side shard + relayout (numpy moves/casts only)."""
    f16 = np.float16
    f32 = np.float32
    gks = np.asarray(inputs["glimpse_K_static"], f32)   # [H,B,1,N,KS]
    gvs = np.asarray(inputs["glimpse_V_static"], f32)
    lks = np.asarray(inputs["logit_K_static"], f32)     # [B,1,N,D]
    ndf = np.asarray(inputs["node_dynamic_features"], f32)  # [B,M,N,8]
    mask = np.asarray(inputs["feasibility_mask"])       # [B,M,N] bool
    prev = np.asarray(inputs["prev_node_embeddings"], f32)  # [B,M,D]
    veh = np.asarray(inputs["vehicle_dynamic_features"], f32)  # [B,M,3]
    fc = np.asarray(inputs["fixed_context"], f32)       # [B,1,D]
    W_pcv = np.asarray(inputs["W_pcv"], f32)            # [D, D+3]
    W_pns = np.asarray(inputs["W_pns"], f32)            # [3D, 8]
    po = np.asarray(inputs["po_weight"], f32)           # [D, D]

    blob = np.zeros((B, 128, BLOBC), dtype=f16)
    # kst: rows (h,k), cols n
    blob[:, :, O_KST:O_KST + N] = (
        gks[:, :, 0].transpose(1, 0, 3, 2).reshape(B, 128, N))
    # lkst: rows d, cols n
    blob[:, :, O_LKST:O_LKST + N] = lks[:, 0].transpose(0, 2, 1)
    # vst: [n%128, (c, (h,k))]
    vpad = np.zeros((B, NPAD, 128), dtype=f16)
    vpad[:, :N, :] = gvs[:, :, 0].transpose(1, 2, 0, 3).reshape(B, N, 128)
    blob[:, :, O_VST:O_VST + NCHUNK * 128] = (
        vpad.reshape(B, NCHUNK, 128, 128).transpose(0, 2, 1, 3)
        .reshape(B, 128, NCHUNK * 128))
    # ndfn: [n%128, (c, (m,f)+ones)]; ones only for real n
    npad = np.zeros((B, NPAD, MF1), dtype=f16)
    npad[:, :N, :MF] = ndf.transpose(0, 2, 1, 3).reshape(B, N, MF)
    npad[:, :N, MF] = 1.0
    blob[:, :, O_NDFN:O_NDFN + NCHUNK * MF1] = (
        npad.reshape(B, NCHUNK, 128, MF1).transpose(0, 2, 1, 3)
        .reshape(B, 128, NCHUNK * MF1))
    # maskT: [n%128, (c, m)] 0/-60000 add-form for the logits path
    mpad = np.full((B, NPAD, M), MASKVAL, dtype=f16)
    mpad[:, :N, :] = np.where(mask, 0.0, MASKVAL).transpose(0, 2, 1)
    blob[:, :, O_MASKT:O_MASKT + NCHUNK * M] = (
        mpad.reshape(B, NCHUNK, 128, M).transpose(0, 2, 1, 3)
        .reshape(B, 128, NCHUNK * M))
    # mask01: [n%128, (c, m)] 0/1 multiply-form for the attention path
    m01 = np.zeros((B, NPAD, M), dtype=f16)
    m01[:, :N, :] = mask.transpose(0, 2, 1).astype(f16)
    blob[:, :, O_MASK01:O_MASK01 + NCHUNK * M] = (
        m01.reshape(B, NCHUNK, 128, M).transpose(0, 2, 1, 3)
        .reshape(B, 128, NCHUNK * M))

    # ndftp: rows 0:40 (m,f) features; two batches per slice (partition
    # bases 0 and 64)
    ndft1 = np.zeros((B, 128, NPAD), dtype=f16)
    ndft1[:, :MF, :N] = ndf.transpose(0, 1, 3, 2).reshape(B, MF, N)

    # constants
    cp16 = np.zeros((128, C16), dtype=f16)
    r8 = np.zeros((8, MF), dtype=f16)
    for m in range(M):
        for ff in range(8):
            r8[ff, m * 8 + ff] = 1.0
    cp16[0:8, C_R8:C_R8 + MF] = r8
    mf_m = np.arange(MF) // 8                      # m of each (m,f) row
    hm_m = np.arange(HM) % M                       # m of each (h,m) col
    cp16[0:MF, C_MASKMT:C_MASKMT + HM] = (
        mf_m[:, None] == hm_m[None, :]).astype(f16)
    cp16[0:MF, C_MASKB5:C_MASKB5 + M] = (
        mf_m[:, None] == np.arange(M)[None, :]).astype(f16)
    bhm_m = np.arange(BHM) % M                     # m of each (b,h,m) col
    cp16[0:MF, C_MASKMA:C_MASKMA + BHM] = (
        mf_m[:, None] == bhm_m[None, :]).astype(f16)
    cp16[0:MF, C_WVSTK:C_WVSTK + 128] = np.tile(
        W_pns[0:128].T.reshape(1, 8, 128), (M, 1, 1)).reshape(MF, 128)
    d_h = np.arange(128) // KS                     # h of each (h,k) row
    hm_h = np.arange(HM) // M                      # h of each (h,m) col
    cp16[:, C_BLKM:C_BLKM + HM] = (
        d_h[:, None] == hm_h[None, :]).astype(f16)
    cp16[:, C_POT:C_POT + 128] = po.T
    cp16[:, C_WL8:C_WL8 + 8] = W_pns[256:384]
    cp16[:, C_PO:C_PO + 128] = po
    cp16[:, C_ONES] = 1.0
    cp16[:, C_WK8:C_WK8 + 8] = W_pns[128:256]

    cp32 = np.zeros((128, F32C), dtype=f32)
    cp32[:, F_WPCVA:F_WPCVA + 128] = W_pcv[:, 0:128].T
    cp32[0:3, F_WPCVB:F_WPCVB + 128] = W_pcv[:, 128:131].T
    sel = np.zeros((HM, 128), dtype=f32)
    for h in range(H):
        sel[h * M:(h + 1) * M, h * KS:(h + 1) * KS] = 1.0
    cp32[0:HM, F_SELT:F_SELT + 128] = sel
    cp32[:, F_ONES] = 1.0
    cp32[0, F_ONER:F_ONER + 128] = 1.0
    cp32[:, F_ONEM:F_ONEM + 128] = 1.0
    cp32[:, F_POT:F_POT + 128] = po.T
    cp32[:, F_PO:F_PO + 128] = po
    cp32[:, F_WL8:F_WL8 + 8] = W_pns[256:384]

    in_maps = []
    for cid in range(NCORES):
        sl = slice(cid * BL, (cid + 1) * BL)
        c32 = cp32.copy()
        # fc replicated per m, prev/veh transposed, cols (b, m)
        c32[:, F_FCM:F_FCM + BM] = np.repeat(
            fc[sl, 0].T, M, axis=1)
        c32[:, F_PREVT:F_PREVT + BM] = (
            prev[sl].transpose(2, 0, 1).reshape(128, BM))
        c32[0:3, F_VEHT:F_VEHT + BM] = (
            veh[sl].transpose(2, 0, 1).reshape(3, BM))
        ndp = np.zeros((BL // 2, 128, NPAD), dtype=f16)
        nd = ndft1[sl]
        ndp[:, 0:MF] = nd[0::2, :MF]
        ndp[:, 64:64 + MF] = nd[1::2, :MF]
        in_maps.append(dict(blob=blob[sl], ndftp=ndp,
                            cp16=cp16.copy(), cp32=c32))
    return in_maps


def kernel(**inputs) -> np.ndarray:
    from concourse import bass_utils

    if "nc" not in _CACHE:
        _CACHE["nc"] = _build_program()
    nc = _CACHE["nc"]
    in_maps = _prep_inputs(inputs)
    res = bass_utils.run_bass_kernel_spmd(nc, in_maps, core_ids=list(range(NCORES)))
    outs = []
    for c in range(NCORES):
        o = res.results[c]["out"]                  # [128, (b, c, m)]
        o = (o.reshape(128, BL, NCHUNK, M).transpose(1, 3, 2, 0)
             .reshape(BL, M, NPAD)[:, :, :N].reshape(BL, M * N))
        outs.append(o)
    return np.concatenate(outs, axis=0).astype(np.float32)


# revision 4
# speedup vs baseline: 1.5660x; 1.1422x over previous
"""Trainium2 Bass kernel for nn_Agent_214748364878 (sparse_attention), v2.

Pure data parallel over batch B=64 -> 8 batches per core. Uses the algebraic
identity  Q . (Kstat + ndf @ Wk) = Q . Kstat + (QWk) . ndf  so the huge
[H,B,M,N,KS] tensors of the reference are never materialized; every big
tensor streams through the PE exactly once.

v2 vs v1: fp16 matmul operands everywhere (1 PE cycle/row instead of 4 for
fp32; validated max rel err 1.5e-3 vs the 2e-2 gate), one fused DMA blob per
batch instead of 7 transfers, n-on-partitions logits phase ([128,40] tiles
instead of [5,512]), prep/tail math batched across the core's 8 batches, and
full cross-batch pipelining (multi-buffered PSUM/SBUF pools, no persistent
in-place tiles on the critical path).

Shapes: B=64, M=5 vehicles, N=1000 nodes, D=128, H=8 heads, KS=16.
Output: softmax probs [64, 5000] (joint softmax over M*N per batch).
"""

import math
import numpy as np

B, M, N, D, H = 64, 5, 1000, 128, 8
KS = D // H
NCORES = 8
BL = B // NCORES          # 8 batches per core
NPAD = 1024
NCHUNK = 8
MF = M * 8                # 40 (m, feature) pairs
HM = H * M                # 40 (head, vehicle) pairs
BM = BL * M               # 40 (batch, vehicle) pairs
BHM = BL * HM             # 320
MASKVAL = -60000.0        # fits fp16; exp underflows to exactly 0

# blob free-dim column offsets (fp16 elements)
O_KST = 0                 # [128=(h,k), 1024=n]
O_LKST = 1024             # [128=d, 1024=n]
O_VST = 2048              # [128=n%128, (c, 128=(h,k))]
O_NDFN = 3072             # [128=n%128, (c, 41=(m,f)+ones)]
O_MASKT = 3400            # [128=n%128, (c, 5=m)]  logits mask, 0/-60000
O_MASK01 = 3440           # [128=n%128, (c, 5=m)]  attention mask, 0/1
MF1 = 41                  # ndfn cols per chunk incl ones
BLOBC = 3480

# cpack16 column offsets (fp16)
C_R8 = 0                  # [8, 40]
C_MASKMT = 40             # [40, 40]  (mf, hm) same-m
C_MASKB5 = 80             # [40, 5]   (mf, m') same-m
C_MASKMA = 85             # [40, 320] (mf, (b,h,m)) same-m
C_WVSTK = 405             # [40, 128] (mf, d) = Wv.T tiled
C_BLKM = 533              # [128, 40] ((h',k), (h,m)) same-h
C_POT = 573               # [128, 128] po.T
C_WL8 = 701               # [128, 8] W_pns[256:384]
C_PO = 709                # [128, 128] po (for on-device po.T @ Wl)
C_ONES = 837              # [128, 1] ones
C_WK8 = 838               # [128, 8] W_pns[128:256]
C16 = 846

# cpack32 column offsets (fp32)
F_WPCVA = 0               # [128, 128] W_pcv[:, :128].T
F_WPCVB = 128             # [3, 128]   W_pcv[:, 128:].T
F_SELT = 256              # [40, 128]  ((h,m), (h',k)) same-h
F_ONEM = 384              # [128, 128] all-ones (partition-sum broadcast)
F_PREVT = 512             # [128, 40]  prev_node_embeddings.T
F_VEHT = 552              # [3, 40]    vehicle_dynamic_features.T
F_FC8 = 592               # [8, 128]   fixed_context rows per batch
F_BSEL2 = 720             # [8, 40]    batch selector [j==b]
F32C = 760

_CACHE = {}


def _build_program():
    import concourse.bass as bass
    import concourse.bacc as bacc
    import concourse.tile as tile
    from concourse import mybir

    f32 = mybir.dt.float32
    f16 = mybir.dt.float16
    nc = bacc.Bacc("TRN2", target_bir_lowering=False, debug=False)

    d_blob = nc.dram_tensor("blob", [BL, 128, BLOBC], f16, kind="ExternalInput")
    d_ndftp = nc.dram_tensor("ndftp", [BL // 2, 128, NPAD], f16,
                             kind="ExternalInput")
    d_cp16 = nc.dram_tensor("cp16", [128, C16], f16, kind="ExternalInput")
    d_cp32 = nc.dram_tensor("cp32", [128, F32C], f32, kind="ExternalInput")
    d_out = nc.dram_tensor("out", [128, BL * NCHUNK * M], f32,
                           kind="ExternalOutput")

    mult = mybir.AluOpType.mult
    add = mybir.AluOpType.add
    EXP = mybir.ActivationFunctionType.Exp
    TANH = mybir.ActivationFunctionType.Tanh

    with tile.TileContext(nc) as tc:
        with (
            tc.tile_pool(name="consts", bufs=1) as consts,
            tc.tile_pool(name="persist", bufs=1) as persist,
            tc.tile_pool(name="dmab", bufs=4) as dmab,
            tc.tile_pool(name="dman", bufs=3) as dman,
            tc.tile_pool(name="work", bufs=3) as work,
            tc.tile_pool(name="ps_ct", bufs=3, space="PSUM") as ps_ct_pool,
            tc.tile_pool(name="ps_ut", bufs=3, space="PSUM") as ps_ut_pool,
            tc.tile_pool(name="ps_tail", bufs=2, space="PSUM") as ps_tail_pool,
        ):
            cp32 = consts.tile([128, F32C], f32)
            nc.scalar.dma_start(cp32[:], d_cp32.ap())
            cp16 = consts.tile([128, C16], f16)
            nc.scalar.dma_start(cp16[:], d_cp16.ap())

            # ---------------- prep phase (once, all 8 batches) ----------------
            ps_q = ps_ct_pool.tile([128, BM], f32, tag="ct")
            nc.tensor.matmul(ps_q[:], cp32[:, F_WPCVA:F_WPCVA + 128],
                             cp32[:, F_PREVT:F_PREVT + BM],
                             start=True, stop=False)
            nc.tensor.matmul(ps_q[:], cp32[0:3, F_WPCVB:F_WPCVB + 128],
                             cp32[0:3, F_VEHT:F_VEHT + BM],
                             start=False, stop=False)
            nc.tensor.matmul(ps_q[:], cp32[0:8, F_FC8:F_FC8 + 128],
                             cp32[0:8, F_BSEL2:F_BSEL2 + BM],
                             start=False, stop=True)
            qT_all = work.tile([128, BM], f32, tag="qT_all")
            nc.vector.tensor_copy(qT_all[:], ps_q[:])

            # block-diag Q, all batches: [128=(h,k), (b,h,m)]
            lhsT1 = persist.tile([128, BHM], f16)
            nc.vector.tensor_tensor(
                lhsT1[:].rearrange("p (b h m) -> p b h m", b=BL, h=H),
                qT_all[:].rearrange("p (b m) -> p b m", b=BL)[:, :, None, :]
                .broadcast_to([128, BL, H, M]),
                cp16[:, C_BLKM:C_BLKM + HM]
                .rearrange("p (h m) -> p h m", h=H)[:, None, :, :]
                .broadcast_to([128, BL, H, M]),
                op=mult)

            # per-head QWk replicated over m' -> lhsT2 rows 0:40; mask-bias
            # selector rows 40:45
            ps_qwk = ps_ct_pool.tile([8, BHM], f32, tag="ct")
            nc.tensor.matmul(ps_qwk[:], cp16[:, C_WK8:C_WK8 + 8], lhsT1[:])
            qwk_sb = work.tile([8, BHM], f16, tag="qwk_sb")
            nc.vector.tensor_copy(qwk_sb[:], ps_qwk[:])
            ps_rep = ps_ct_pool.tile([MF, BHM], f32, tag="ct")
            nc.tensor.matmul(ps_rep[:], cp16[0:8, C_R8:C_R8 + MF], qwk_sb[:])
            # duplicated at partition bases 0 and 64 to pair with the
            # two-batches-per-tile ndft layout (matmul requires equal
            # base partitions for lhsT and rhs)
            lhsT2 = persist.tile([64 + MF, BHM], f16)
            for nb in (0, 64):
                nc.vector.tensor_tensor(lhsT2[nb:nb + MF, :], ps_rep[:],
                                        cp16[0:MF, C_MASKMA:C_MASKMA + BHM],
                                        op=mult)

            # powl = po.T @ Wl (on-device weight fusion for the logits
            # dynamic path; lets FWl come straight from concT, parallel to fq)
            ps_powl = ps_tail_pool.tile([128, 8], f32, tag="tail")
            nc.tensor.matmul(ps_powl[:], cp16[:, C_PO:C_PO + 128],
                             cp16[:, C_WL8:C_WL8 + 8])
            powl = persist.tile([128, 8], f16)
            nc.vector.tensor_copy(powl[:], ps_powl[:])

            # ------------- per-batch pipeline, software-pipelined -------------
            out_all = persist.tile([128, BL * NCHUNK * M], f32)
            ndft_tiles = {}
            state = {}

            def stage_a(b):
                """DMA in + compat + exp + feasibility mask."""
                blob = dmab.tile([128, BLOBC], f16, tag="blob")
                # split by consumer stage: kst (compat) first, then
                # vst/ndfn/masks (acc + masks), then lkst (logits);
                # alternate issue queues to overlap DGE pipelines
                eng = nc.sync if b % 2 == 0 else nc.scalar
                eng.dma_start(blob[:, 0:1024], d_blob.ap()[b][:, 0:1024])
                eng.dma_start(blob[:, 2048:BLOBC],
                              d_blob.ap()[b][:, 2048:BLOBC])
                eng.dma_start(blob[:, 1024:2048],
                              d_blob.ap()[b][:, 1024:2048])
                if b % 2 == 0:
                    nd = dman.tile([128, NPAD], f16, tag="ndft")
                    nc.sync.dma_start(nd[:], d_ndftp.ap()[b // 2])
                    ndft_tiles[b // 2] = nd
                nb = 64 * (b % 2)
                ndft_t = ndft_tiles[b // 2]
                ps_ct = ps_ct_pool.tile([128, NCHUNK * HM], f32, tag="ct")
                for c in range(NCHUNK):
                    cs = slice(c * HM, (c + 1) * HM)
                    nc.tensor.matmul(ps_ct[:, cs],
                                     blob[:, O_KST + c * 128:O_KST + (c + 1) * 128],
                                     lhsT1[:, b * HM:(b + 1) * HM],
                                     start=True, stop=False)
                    nc.tensor.matmul(ps_ct[:, cs],
                                     ndft_t[nb:nb + MF, c * 128:(c + 1) * 128],
                                     lhsT2[nb:nb + MF, b * HM:(b + 1) * HM],
                                     start=False, stop=True)
                ETu = work.tile([128, NCHUNK * HM], f16, tag="ETu")
                nc.scalar.activation(ETu[:], ps_ct[:], EXP, scale=0.25)
                # 0/1 feasibility mask post-exp; Z/U/S all consume the masked
                # E so this is exact
                ET = work.tile([128, NCHUNK * HM], f16, tag="ET")
                nc.vector.tensor_tensor(
                    ET[:].rearrange("p (c h m) -> p c h m", c=NCHUNK, h=H),
                    ETu[:].rearrange("p (c h m) -> p c h m", c=NCHUNK, h=H),
                    blob[:, O_MASK01:O_MASK01 + NCHUNK * M]
                    .rearrange("p (c m) -> p c m", c=NCHUNK)[:, :, None, :]
                    .broadcast_to([128, NCHUNK, H, M]),
                    op=mult)
                state[b] = dict(blob=blob, ndft=ndft_t, nb=nb, ET=ET)

            def stage_b(p):
                """S/U/Z accumulation + attention tail for batch pair
                (2p, 2p+1) — pairing halves the per-batch chain length."""
                b0, b1 = 2 * p, 2 * p + 1
                st0, st1 = state[b0], state[b1]
                # ps_st: per-batch S^T (cols 0:40 / 41:81) + Z (cols 40 / 81)
                # ps_ut: per-batch U^T side by side [128, 80].
                # One spanning accumulation group per bank (first matmul
                # start=True covers the zero region; last has stop=True).
                tailt = ps_tail_pool.tile([128, 196], f32, tag="tail")
                ps_st = tailt[0:MF, 114:196]
                ps_ut = ps_ut_pool.tile([128, 2 * HM], f32, tag="ut")
                for i, st in ((0, st0), (1, st1)):
                    blob, ET = st["blob"], st["ET"]
                    so, uo = 41 * i, HM * i
                    for c in range(NCHUNK):
                        cs = slice(c * HM, (c + 1) * HM)
                        nc.tensor.matmul(ps_st[:, so:so + 40],
                                         blob[:, O_NDFN + c * MF1:O_NDFN + c * MF1 + MF],
                                         ET[:, cs],
                                         start=(c == 0 and i == 0), stop=False,
                                         skip_group_check=True)
                        nc.tensor.matmul(ps_st[:, so + 40:so + 41], ET[:, cs],
                                         cp16[:, C_ONES:C_ONES + 1],
                                         start=False, stop=False,
                                         skip_group_check=True)
                        nc.tensor.matmul(ps_ut[:, uo:uo + HM],
                                         blob[:, O_VST + c * 128:O_VST + (c + 1) * 128],
                                         ET[:, cs],
                                         start=(c == 0 and i == 0), stop=False,
                                         skip_group_check=True)
                # U2^T for both batches via masked S^T
                SmT = work.tile([MF, 2 * HM], f16, tag="SmT")
                nc.vector.tensor_tensor(
                    SmT[:].rearrange("p (i x) -> p i x", i=2),
                    ps_st[:].rearrange("p (i x) -> p i x", i=2)[:, :, 0:40],
                    cp16[0:MF, C_MASKMT:C_MASKMT + HM][:, None, :]
                    .broadcast_to([MF, 2, HM]),
                    op=mult)
                nc.tensor.matmul(ps_ut[:, 0:HM],
                                 cp16[0:MF, C_WVSTK:C_WVSTK + 128],
                                 SmT[:, 0:HM], start=False, stop=False,
                                 skip_group_check=True)
                nc.tensor.matmul(ps_ut[:, HM:2 * HM],
                                 cp16[0:MF, C_WVSTK:C_WVSTK + 128],
                                 SmT[:, HM:2 * HM], start=False, stop=True,
                                 skip_group_check=True)
                # per-head 1/Z for both batches in one matmul/reciprocal
                r40 = work.tile([MF, 2], f32, tag="r40")
                nc.vector.tensor_copy(
                    r40[:], ps_st[:].rearrange("p (i x) -> p i x", i=2)[:, :, 40])
                nc.tensor.matmul(tailt[:, 90:92],
                                 cp32[0:HM, F_SELT:F_SELT + 128], r40[:],
                                 skip_group_check=True)
                zinv = work.tile([128, 2], f32, tag="zinv")
                nc.vector.reciprocal(zinv[:], tailt[:, 90:92])
                # masked normalized U^T -> fq, FWl (accumulated over heads)
                utm = work.tile([128, 2 * HM], f16, tag="utm")
                for i in (0, 1):
                    nc.vector.scalar_tensor_tensor(
                        utm[:, HM * i:HM * (i + 1)],
                        ps_ut[:, HM * i:HM * (i + 1)], zinv[:, i:i + 1],
                        cp16[:, C_BLKM:C_BLKM + HM], op0=mult, op1=mult)
                for i in (0, 1):
                    for h in range(H):
                        hs = slice(HM * i + h * M, HM * i + (h + 1) * M)
                        nc.tensor.matmul(tailt[:, 80 + 5 * i:85 + 5 * i],
                                         cp16[:, C_POT:C_POT + 128], utm[:, hs],
                                         start=(h == 0 and i == 0), stop=False,
                                         skip_group_check=True)
                        nc.tensor.matmul(tailt[0:8, 94 + 5 * i:99 + 5 * i],
                                         powl[:], utm[:, hs],
                                         start=False,
                                         stop=(h == H - 1 and i == 1),
                                         skip_group_check=True)
                fqT = work.tile([128, 2 * M], f16, tag="fqT")
                nc.scalar.activation(fqT[:], tailt[:, 80:90],
                                     mybir.ActivationFunctionType.Copy)
                fwl = work.tile([8, 2 * M], f16, tag="fwl")
                nc.vector.tensor_copy(fwl[:], tailt[0:8, 94:104])
                nc.tensor.matmul(tailt[0:MF, 104:114],
                                 cp16[0:8, C_R8:C_R8 + MF], fwl[:],
                                 skip_group_check=True)
                lhsT3 = work.tile([64 + MF, 2 * M], f16, tag="lhsT3")
                for i in (0, 1):
                    nb = 64 * i
                    nc.vector.tensor_tensor(
                        lhsT3[nb:nb + MF, 5 * i:5 * (i + 1)],
                        tailt[0:MF, 104 + 5 * i:109 + 5 * i],
                        cp16[0:MF, C_MASKB5:C_MASKB5 + M],
                        op=mult)
                st0["tailt"] = st1["tailt"] = tailt
                st0["fqT"] = st1["fqT"] = fqT
                st0["lhsT3"] = st1["lhsT3"] = lhsT3

            def stage_c(p):
                """Logits + joint softmax + output for batch pair."""
                b0, b1 = 2 * p, 2 * p + 1
                st0, st1 = state.pop(b0), state.pop(b1)
                tailt = st0["tailt"]
                fqT, lhsT3 = st0["fqT"], st0["lhsT3"]
                for i, st in ((0, st0), (1, st1)):
                    blob, ndft_t, nb = st["blob"], st["ndft"], st["nb"]
                    for c in range(NCHUNK):
                        cs = slice(40 * i + c * M, 40 * i + (c + 1) * M)
                        nc.tensor.matmul(tailt[:, cs],
                                         blob[:, O_LKST + c * 128:O_LKST + (c + 1) * 128],
                                         fqT[:, 5 * i:5 * (i + 1)],
                                         start=True, stop=False,
                                         skip_group_check=True)
                        nc.tensor.matmul(tailt[:, cs],
                                         ndft_t[nb:nb + MF, c * 128:(c + 1) * 128],
                                         lhsT3[nb:nb + MF, 5 * i:5 * (i + 1)],
                                         start=False, stop=True,
                                         skip_group_check=True)
                tl = work.tile([128, 2 * NCHUNK * M], f32, tag="tl")
                nc.scalar.activation(tl[:], tailt[:, 0:80], TANH,
                                     scale=1.0 / math.sqrt(D))
                pl = work.tile([128, 2 * NCHUNK * M], f32, tag="pl")
                for i, st in ((0, st0), (1, st1)):
                    nc.vector.scalar_tensor_tensor(
                        pl[:, 40 * i:40 * (i + 1)], tl[:, 40 * i:40 * (i + 1)],
                        10.0, st["blob"][:, O_MASKT:O_MASKT + 40],
                        op0=mult, op1=add)
                eL = work.tile([128, 2 * NCHUNK * M], f32, tag="eL")
                rL = work.tile([128, 2], f32, tag="rL")
                nc.scalar.activation(eL[:, 0:40], pl[:, 0:40], EXP,
                                     accum_out=rL[:, 0:1])
                nc.scalar.activation(eL[:, 40:80], pl[:, 40:80], EXP,
                                     accum_out=rL[:, 1:2])
                nc.tensor.matmul(tailt[:, 92:94],
                                 cp32[:, F_ONEM:F_ONEM + 128], rL[:],
                                 skip_group_check=True)
                zbinv = work.tile([128, 2], f32, tag="zbinv")
                nc.vector.reciprocal(zbinv[:], tailt[:, 92:94])
                for i, b in ((0, b0), (1, b1)):
                    nc.vector.tensor_scalar_mul(
                        out_all[:, b * NCHUNK * M:(b + 1) * NCHUNK * M],
                        eL[:, 40 * i:40 * (i + 1)], zbinv[:, i:i + 1])

            import os
            STAGES = int(os.environ.get("KV2_STAGES", "3"))
            LAG = int(os.environ.get("KV2_LAG", "1"))
            ORDER = os.environ.get("KV2_ORDER", "abc")
            _stage_map = _CACHE.setdefault("stage_map", [])

            def _mark(tag, fn, *a):
                i0 = nc.next_id()
                fn(*a)
                _stage_map.append((tag, i0, nc.next_id()))

            for t in range(BL + 4):
                for s in ORDER:
                    if s == "a" and t < BL:
                        _mark(f"A{t}", stage_a, t)
                    if (s == "b" and STAGES >= 2 and t % 2 == 1
                            and 0 <= (t - 1) // 2 < BL // 2):
                        _mark(f"B{(t - 1) // 2}", stage_b, (t - 1) // 2)
                    if (s == "c" and STAGES >= 3 and t % 2 == 0
                            and 0 <= (t - 4) // 2 < BL // 2):
                        _mark(f"C{(t - 4) // 2}", stage_c, (t - 4) // 2)
            if STAGES < 3:
                nc.vector.memset(out_all[:], 0.0)
            nc.sync.dma_start(d_out.ap(), out_all[:])

    nc.compile()
    return nc


def _prep_inputs(inputs):
    """Host-side shard + relayout (numpy moves/casts only)."""
    f16 = np.float16
    f32 = np.float32
    gks = np.asarray(inputs["glimpse_K_static"], f32)   # [H,B,1,N,KS]
    gvs = np.asarray(inputs["glimpse_V_static"], f32)
    lks = np.asarray(inputs["logit_K_static"], f32)     # [B,1,N,D]
    ndf = np.asarray(inputs["node_dynamic_features"], f32)  # [B,M,N,8]
    mask = np.asarray(inputs["feasibility_mask"])       # [B,M,N] bool
    prev = np.asarray(inputs["prev_node_embeddings"], f32)  # [B,M,D]
    veh = np.asarray(inputs["vehicle_dynamic_features"], f32)  # [B,M,3]
    fc = np.asarray(inputs["fixed_context"], f32)       # [B,1,D]
    W_pcv = np.asarray(inputs["W_pcv"], f32)            # [D, D+3]
    W_pns = np.asarray(inputs["W_pns"], f32)            # [3D, 8]
    po = np.asarray(inputs["po_weight"], f32)           # [D, D]

    blob = np.zeros((B, 128, BLOBC), dtype=f16)
    # kst: rows (h,k), cols n
    blob[:, :, O_KST:O_KST + N] = (
        gks[:, :, 0].transpose(1, 0, 3, 2).reshape(B, 128, N))
    # lkst: rows d, cols n
    blob[:, :, O_LKST:O_LKST + N] = lks[:, 0].transpose(0, 2, 1)
    # vst: [n%128, (c, (h,k))]
    vpad = np.zeros((B, NPAD, 128), dtype=f16)
    vpad[:, :N, :] = gvs[:, :, 0].transpose(1, 2, 0, 3).reshape(B, N, 128)
    blob[:, :, O_VST:O_VST + NCHUNK * 128] = (
        vpad.reshape(B, NCHUNK, 128, 128).transpose(0, 2, 1, 3)
        .reshape(B, 128, NCHUNK * 128))
    # ndfn: [n%128, (c, (m,f)+ones)]; ones only for real n
    npad = np.zeros((B, NPAD, MF1), dtype=f16)
    npad[:, :N, :MF] = ndf.transpose(0, 2, 1, 3).reshape(B, N, MF)
    npad[:, :N, MF] = 1.0
    blob[:, :, O_NDFN:O_NDFN + NCHUNK * MF1] = (
        npad.reshape(B, NCHUNK, 128, MF1).transpose(0, 2, 1, 3)
        .reshape(B, 128, NCHUNK * MF1))
    # maskT: [n%128, (c, m)] 0/-60000 add-form for the logits path
    mpad = np.full((B, NPAD, M), MASKVAL, dtype=f16)
    mpad[:, :N, :] = np.where(mask, 0.0, MASKVAL).transpose(0, 2, 1)
    blob[:, :, O_MASKT:O_MASKT + NCHUNK * M] = (
        mpad.reshape(B, NCHUNK, 128, M).transpose(0, 2, 1, 3)
        .reshape(B, 128, NCHUNK * M))
    # mask01: [n%128, (c, m)] 0/1 multiply-form for the attention path
    m01 = np.zeros((B, NPAD, M), dtype=f16)
    m01[:, :N, :] = mask.transpose(0, 2, 1).astype(f16)
    blob[:, :, O_MASK01:O_MASK01 + NCHUNK * M] = (
        m01.reshape(B, NCHUNK, 128, M).transpose(0, 2, 1, 3)
        .reshape(B, 128, NCHUNK * M))

    # ndftp: rows 0:40 (m,f) features; two batches per slice (partition
    # bases 0 and 64)
    ndft1 = np.zeros((B, 128, NPAD), dtype=f16)
    ndft1[:, :MF, :N] = ndf.transpose(0, 1, 3, 2).reshape(B, MF, N)

    # constants
    cp16 = np.zeros((128, C16), dtype=f16)
    r8 = np.zeros((8, MF), dtype=f16)
    for m in range(M):
        for ff in range(8):
            r8[ff, m * 8 + ff] = 1.0
    cp16[0:8, C_R8:C_R8 + MF] = r8
    mf_m = np.arange(MF) // 8                      # m of each (m,f) row
    hm_m = np.arange(HM) % M                       # m of each (h,m) col
    cp16[0:MF, C_MASKMT:C_MASKMT + HM] = (
        mf_m[:, None] == hm_m[None, :]).astype(f16)
    cp16[0:MF, C_MASKB5:C_MASKB5 + M] = (
        mf_m[:, None] == np.arange(M)[None, :]).astype(f16)
    bhm_m = np.arange(BHM) % M                     # m of each (b,h,m) col
    cp16[0:MF, C_MASKMA:C_MASKMA + BHM] = (
        mf_m[:, None] == bhm_m[None, :]).astype(f16)
    cp16[0:MF, C_WVSTK:C_WVSTK + 128] = np.tile(
        W_pns[0:128].T.reshape(1, 8, 128), (M, 1, 1)).reshape(MF, 128)
    d_h = np.arange(128) // KS                     # h of each (h,k) row
    hm_h = np.arange(HM) // M                      # h of each (h,m) col
    cp16[:, C_BLKM:C_BLKM + HM] = (
        d_h[:, None] == hm_h[None, :]).astype(f16)
    cp16[:, C_POT:C_POT + 128] = po.T
    cp16[:, C_WL8:C_WL8 + 8] = W_pns[256:384]
    cp16[:, C_PO:C_PO + 128] = po
    cp16[:, C_ONES] = 1.0
    cp16[:, C_WK8:C_WK8 + 8] = W_pns[128:256]

    cp32 = np.zeros((128, F32C), dtype=f32)
    cp32[:, F_WPCVA:F_WPCVA + 128] = W_pcv[:, 0:128].T
    cp32[0:3, F_WPCVB:F_WPCVB + 128] = W_pcv[:, 128:131].T
    sel = np.zeros((HM, 128), dtype=f32)
    for h in range(H):
        sel[h * M:(h + 1) * M, h * KS:(h + 1) * KS] = 1.0
    cp32[0:HM, F_SELT:F_SELT + 128] = sel
    cp32[:, F_ONEM:F_ONEM + 128] = 1.0

    in_maps = []
    for cid in range(NCORES):
        sl = slice(cid * BL, (cid + 1) * BL)
        c32 = cp32.copy()
        # fc rows per batch + batch selector, prev/veh transposed
        c32[0:BL, F_FC8:F_FC8 + 128] = fc[sl, 0]
        c32[0:BL, F_BSEL2:F_BSEL2 + BM] = (
            np.arange(BL)[:, None] == (np.arange(BM) // M)[None, :])
        c32[:, F_PREVT:F_PREVT + BM] = (
            prev[sl].transpose(2, 0, 1).reshape(128, BM))
        c32[0:3, F_VEHT:F_VEHT + BM] = (
            veh[sl].transpose(2, 0, 1).reshape(3, BM))
        ndp = np.zeros((BL // 2, 128, NPAD), dtype=f16)
        nd = ndft1[sl]
        ndp[:, 0:MF] = nd[0::2, :MF]
        ndp[:, 64:64 + MF] = nd[1::2, :MF]
        in_maps.append(dict(blob=blob[sl], ndftp=ndp,
                            cp16=cp16.copy(), cp32=c32))
    return in_maps


def kernel(**inputs) -> np.ndarray:
    from concourse import bass_utils

    if "nc" not in _CACHE:
        _CACHE["nc"] = _build_program()
    nc = _CACHE["nc"]
    in_maps = _prep_inputs(inputs)
    res = bass_utils.run_bass_kernel_spmd(nc, in_maps, core_ids=list(range(NCORES)))
    outs = []
    for c in range(NCORES):
        o = res.results[c]["out"]                  # [128, (b, c, m)]
        o = (o.reshape(128, BL, NCHUNK, M).transpose(1, 3, 2, 0)
             .reshape(BL, M, NPAD)[:, :, :N].reshape(BL, M * N))
        outs.append(o)
    return np.concatenate(outs, axis=0).astype(np.float32)


# revision 5
# speedup vs baseline: 1.6305x; 1.0412x over previous
"""Trainium2 Bass kernel for nn_Agent_214748364878 (sparse_attention), v2.

Pure data parallel over batch B=64 -> 8 batches per core. Uses the algebraic
identity  Q . (Kstat + ndf @ Wk) = Q . Kstat + (QWk) . ndf  so the huge
[H,B,M,N,KS] tensors of the reference are never materialized; every big
tensor streams through the PE exactly once.

v2 vs v1: fp16 matmul operands everywhere (1 PE cycle/row instead of 4 for
fp32; validated max rel err 1.5e-3 vs the 2e-2 gate), one fused DMA blob per
batch instead of 7 transfers, n-on-partitions logits phase ([128,40] tiles
instead of [5,512]), prep/tail math batched across the core's 8 batches, and
full cross-batch pipelining (multi-buffered PSUM/SBUF pools, no persistent
in-place tiles on the critical path).

Shapes: B=64, M=5 vehicles, N=1000 nodes, D=128, H=8 heads, KS=16.
Output: softmax probs [64, 5000] (joint softmax over M*N per batch).
"""

import math
import numpy as np

B, M, N, D, H = 64, 5, 1000, 128, 8
KS = D // H
NCORES = 8
BL = B // NCORES          # 8 batches per core
NPAD = 1024
NCHUNK = 8
MF = M * 8                # 40 (m, feature) pairs
HM = H * M                # 40 (head, vehicle) pairs
BM = BL * M               # 40 (batch, vehicle) pairs
BHM = BL * HM             # 320
MASKVAL = -60000.0        # fits fp16; exp underflows to exactly 0

# blob free-dim column offsets (fp16 elements)
O_KST = 0                 # [128=(h,k), 1024=n]
O_LKST = 1024             # [128=d, 1024=n]
O_VST = 2048              # [128=n%128, (c, 128=(h,k))]
O_NDFN = 3072             # [128=n%128, (c, 41=(m,f)+ones)]
O_MASKT = 3400            # [128=n%128, (c, 5=m)]  logits mask, 0/-60000
O_MASK01 = 3440           # [128=n%128, (c, 5=m)]  attention mask, 0/1
MF1 = 41                  # ndfn cols per chunk incl ones
BLOBC = 3480

# cpack16 column offsets (fp16); prep-critical regions first so the
# first (split) DMA unblocks the prep chain early
C_R8 = 0                  # [8, 40]
C_MASKMA = 40             # [40, 320] (mf, (b,h,m)) same-m
C_BLKM = 360              # [128, 40] ((h',k), (h,m)) same-h
C_WK8 = 400               # [128, 8] W_pns[128:256]
C16A = 408                # prep/rest boundary
C_MASKMT = 408            # [40, 40]  (mf, hm) same-m
C_MASKB5 = 448            # [40, 5]   (mf, m') same-m
C_WVSTK = 453             # [40, 128] (mf, d) = Wv.T tiled
C_POT = 581               # [128, 128] po.T
C_WL8 = 709               # [128, 8] W_pns[256:384]
C_PO = 717                # [128, 128] po (for on-device po.T @ Wl)
C_ONES = 845              # [128, 1] ones
C16 = 846

# cpack32 column offsets (fp32); prep-critical regions first
F_WPCVA = 0               # [128, 128] W_pcv[:, :128].T
F_WPCVB = 128             # [3, 128]   W_pcv[:, 128:].T
F_PREVT = 256             # [128, 40]  prev_node_embeddings.T
F_VEHT = 296              # [3, 40]    vehicle_dynamic_features.T
F_FC8 = 336               # [8, 128]   fixed_context rows per batch
F_BSEL2 = 464             # [8, 40]    batch selector [j==b]
F32A = 504                # prep/rest boundary
F_SELT = 504              # [40, 128]  ((h,m), (h',k)) same-h
F_ONEM = 632              # [128, 128] all-ones (partition-sum broadcast)
F32C = 760

_CACHE = {}


def _build_program():
    import concourse.bass as bass
    import concourse.bacc as bacc
    import concourse.tile as tile
    from concourse import mybir

    f32 = mybir.dt.float32
    f16 = mybir.dt.float16
    nc = bacc.Bacc("TRN2", target_bir_lowering=False, debug=False)

    d_blob = nc.dram_tensor("blob", [BL, 128, BLOBC], f16, kind="ExternalInput")
    d_ndftp = nc.dram_tensor("ndftp", [BL // 2, 128, NPAD], f16,
                             kind="ExternalInput")
    d_cp16 = nc.dram_tensor("cp16", [128, C16], f16, kind="ExternalInput")
    d_cp32 = nc.dram_tensor("cp32", [128, F32C], f32, kind="ExternalInput")
    d_out = nc.dram_tensor("out", [128, BL * NCHUNK * M], f32,
                           kind="ExternalOutput")

    mult = mybir.AluOpType.mult
    add = mybir.AluOpType.add
    EXP = mybir.ActivationFunctionType.Exp
    TANH = mybir.ActivationFunctionType.Tanh

    with tile.TileContext(nc) as tc:
        with (
            tc.tile_pool(name="consts", bufs=1) as consts,
            tc.tile_pool(name="persist", bufs=1) as persist,
            tc.tile_pool(name="dmab", bufs=4) as dmab,
            tc.tile_pool(name="dman", bufs=3) as dman,
            tc.tile_pool(name="work", bufs=3) as work,
            tc.tile_pool(name="ps_ct", bufs=3, space="PSUM") as ps_ct_pool,
            tc.tile_pool(name="ps_ut", bufs=3, space="PSUM") as ps_ut_pool,
            tc.tile_pool(name="ps_tail", bufs=2, space="PSUM") as ps_tail_pool,
        ):
            cp32 = consts.tile([128, F32C], f32)
            cp16 = consts.tile([128, C16], f16)
            nc.scalar.dma_start(cp32[:], d_cp32.ap())
            nc.scalar.dma_start(cp16[:], d_cp16.ap())

            # ---------------- prep phase (once, all 8 batches) ----------------
            ps_q = ps_ct_pool.tile([128, BM], f32, tag="ct")
            nc.tensor.matmul(ps_q[:], cp32[:, F_WPCVA:F_WPCVA + 128],
                             cp32[:, F_PREVT:F_PREVT + BM],
                             start=True, stop=False)
            nc.tensor.matmul(ps_q[:], cp32[0:3, F_WPCVB:F_WPCVB + 128],
                             cp32[0:3, F_VEHT:F_VEHT + BM],
                             start=False, stop=False)
            nc.tensor.matmul(ps_q[:], cp32[0:8, F_FC8:F_FC8 + 128],
                             cp32[0:8, F_BSEL2:F_BSEL2 + BM],
                             start=False, stop=True)
            qT_all = work.tile([128, BM], f32, tag="qT_all")
            nc.vector.tensor_copy(qT_all[:], ps_q[:])

            # block-diag Q, all batches: [128=(h,k), (b,h,m)]
            lhsT1 = persist.tile([128, BHM], f16)
            nc.vector.tensor_tensor(
                lhsT1[:].rearrange("p (b h m) -> p b h m", b=BL, h=H),
                qT_all[:].rearrange("p (b m) -> p b m", b=BL)[:, :, None, :]
                .broadcast_to([128, BL, H, M]),
                cp16[:, C_BLKM:C_BLKM + HM]
                .rearrange("p (h m) -> p h m", h=H)[:, None, :, :]
                .broadcast_to([128, BL, H, M]),
                op=mult)

            # per-head QWk replicated over m' -> lhsT2 rows 0:40; mask-bias
            # selector rows 40:45
            ps_qwk = ps_ct_pool.tile([8, BHM], f32, tag="ct")
            nc.tensor.matmul(ps_qwk[:], cp16[:, C_WK8:C_WK8 + 8], lhsT1[:])
            qwk_sb = work.tile([8, BHM], f16, tag="qwk_sb")
            nc.vector.tensor_copy(qwk_sb[:], ps_qwk[:])
            ps_rep = ps_ct_pool.tile([MF, BHM], f32, tag="ct")
            nc.tensor.matmul(ps_rep[:], cp16[0:8, C_R8:C_R8 + MF], qwk_sb[:])
            # duplicated at partition bases 0 and 64 to pair with the
            # two-batches-per-tile ndft layout (matmul requires equal
            # base partitions for lhsT and rhs)
            lhsT2 = persist.tile([64 + MF, BHM], f16)
            for nb in (0, 64):
                nc.vector.tensor_tensor(lhsT2[nb:nb + MF, :], ps_rep[:],
                                        cp16[0:MF, C_MASKMA:C_MASKMA + BHM],
                                        op=mult)

            # powl = po.T @ Wl (on-device weight fusion for the logits
            # dynamic path; lets FWl come straight from concT, parallel to fq)
            ps_powl = ps_tail_pool.tile([128, 8], f32, tag="tail")
            nc.tensor.matmul(ps_powl[:], cp16[:, C_PO:C_PO + 128],
                             cp16[:, C_WL8:C_WL8 + 8])
            powl = persist.tile([128, 8], f16)
            nc.vector.tensor_copy(powl[:], ps_powl[:])

            # ------------- per-batch pipeline, software-pipelined -------------
            out_all = persist.tile([128, BL * NCHUNK * M], f32)
            ndft_tiles = {}
            state = {}

            def stage_a(b):
                """DMA in + compat + exp + feasibility mask."""
                blob = dmab.tile([128, BLOBC], f16, tag="blob")
                # split by consumer stage: kst (compat) first, then
                # vst/ndfn/masks (acc + masks), then lkst (logits);
                # alternate issue queues to overlap DGE pipelines
                eng = nc.sync if b % 2 == 0 else nc.scalar
                eng.dma_start(blob[:, 0:1024], d_blob.ap()[b][:, 0:1024])
                eng.dma_start(blob[:, 2048:BLOBC],
                              d_blob.ap()[b][:, 2048:BLOBC])
                eng.dma_start(blob[:, 1024:2048],
                              d_blob.ap()[b][:, 1024:2048])
                if b % 2 == 0:
                    nd = dman.tile([128, NPAD], f16, tag="ndft")
                    nc.sync.dma_start(nd[:], d_ndftp.ap()[b // 2])
                    ndft_tiles[b // 2] = nd
                nb = 64 * (b % 2)
                ndft_t = ndft_tiles[b // 2]
                # one spanning accumulation group over the bank: all static
                # matmuls first (they need only lhsT1, ready early in the
                # fill phase), then all dynamic ones (need lhsT2) -- avoids
                # per-chunk head-of-line blocking on the PE stream
                ps_ct = ps_ct_pool.tile([128, NCHUNK * HM], f32, tag="ct")
                for c in range(NCHUNK):
                    cs = slice(c * HM, (c + 1) * HM)
                    nc.tensor.matmul(ps_ct[:, cs],
                                     blob[:, O_KST + c * 128:O_KST + (c + 1) * 128],
                                     lhsT1[:, b * HM:(b + 1) * HM],
                                     start=(c == 0), stop=False,
                                     skip_group_check=True)
                for c in range(NCHUNK):
                    cs = slice(c * HM, (c + 1) * HM)
                    nc.tensor.matmul(ps_ct[:, cs],
                                     ndft_t[nb:nb + MF, c * 128:(c + 1) * 128],
                                     lhsT2[nb:nb + MF, b * HM:(b + 1) * HM],
                                     start=False, stop=(c == NCHUNK - 1),
                                     skip_group_check=True)
                ETu = work.tile([128, NCHUNK * HM], f16, tag="ETu")
                nc.scalar.activation(ETu[:], ps_ct[:], EXP, scale=0.25)
                # 0/1 feasibility mask post-exp; Z/U/S all consume the masked
                # E so this is exact
                ET = work.tile([128, NCHUNK * HM], f16, tag="ET")
                nc.vector.tensor_tensor(
                    ET[:].rearrange("p (c h m) -> p c h m", c=NCHUNK, h=H),
                    ETu[:].rearrange("p (c h m) -> p c h m", c=NCHUNK, h=H),
                    blob[:, O_MASK01:O_MASK01 + NCHUNK * M]
                    .rearrange("p (c m) -> p c m", c=NCHUNK)[:, :, None, :]
                    .broadcast_to([128, NCHUNK, H, M]),
                    op=mult)
                state[b] = dict(blob=blob, ndft=ndft_t, nb=nb, ET=ET)

            def stage_b(p):
                """S/U/Z accumulation + attention tail for batch pair
                (2p, 2p+1) — pairing halves the per-batch chain length."""
                b0, b1 = 2 * p, 2 * p + 1
                st0, st1 = state[b0], state[b1]
                # ps_st: per-batch S^T (cols 0:40 / 41:81) + Z (cols 40 / 81)
                # ps_ut: per-batch U^T side by side [128, 80].
                # One spanning accumulation group per bank (first matmul
                # start=True covers the zero region; last has stop=True).
                tailt = ps_tail_pool.tile([128, 196], f32, tag="tail")
                ps_st = tailt[0:MF, 114:196]
                ps_ut = ps_ut_pool.tile([128, 2 * HM], f32, tag="ut")
                for i, st in ((0, st0), (1, st1)):
                    blob, ET = st["blob"], st["ET"]
                    so, uo = 41 * i, HM * i
                    for c in range(NCHUNK):
                        cs = slice(c * HM, (c + 1) * HM)
                        nc.tensor.matmul(ps_st[:, so:so + 40],
                                         blob[:, O_NDFN + c * MF1:O_NDFN + c * MF1 + MF],
                                         ET[:, cs],
                                         start=(c == 0 and i == 0), stop=False,
                                         skip_group_check=True)
                        nc.tensor.matmul(ps_st[:, so + 40:so + 41], ET[:, cs],
                                         cp16[:, C_ONES:C_ONES + 1],
                                         start=False, stop=False,
                                         skip_group_check=True)
                        nc.tensor.matmul(ps_ut[:, uo:uo + HM],
                                         blob[:, O_VST + c * 128:O_VST + (c + 1) * 128],
                                         ET[:, cs],
                                         start=(c == 0 and i == 0), stop=False,
                                         skip_group_check=True)
                # U2^T for both batches via masked S^T
                SmT = work.tile([MF, 2 * HM], f16, tag="SmT")
                nc.vector.tensor_tensor(
                    SmT[:].rearrange("p (i x) -> p i x", i=2),
                    ps_st[:].rearrange("p (i x) -> p i x", i=2)[:, :, 0:40],
                    cp16[0:MF, C_MASKMT:C_MASKMT + HM][:, None, :]
                    .broadcast_to([MF, 2, HM]),
                    op=mult)
                nc.tensor.matmul(ps_ut[:, 0:HM],
                                 cp16[0:MF, C_WVSTK:C_WVSTK + 128],
                                 SmT[:, 0:HM], start=False, stop=False,
                                 skip_group_check=True)
                nc.tensor.matmul(ps_ut[:, HM:2 * HM],
                                 cp16[0:MF, C_WVSTK:C_WVSTK + 128],
                                 SmT[:, HM:2 * HM], start=False, stop=True,
                                 skip_group_check=True)
                # per-head 1/Z for both batches in one matmul/reciprocal
                r40 = work.tile([MF, 2], f32, tag="r40")
                nc.vector.tensor_copy(
                    r40[:], ps_st[:].rearrange("p (i x) -> p i x", i=2)[:, :, 40])
                nc.tensor.matmul(tailt[:, 90:92],
                                 cp32[0:HM, F_SELT:F_SELT + 128], r40[:],
                                 skip_group_check=True)
                zinv = work.tile([128, 2], f32, tag="zinv")
                nc.vector.reciprocal(zinv[:], tailt[:, 90:92])
                # masked normalized U^T -> fq, FWl (accumulated over heads)
                utm = work.tile([128, 2 * HM], f16, tag="utm")
                for i in (0, 1):
                    nc.vector.scalar_tensor_tensor(
                        utm[:, HM * i:HM * (i + 1)],
                        ps_ut[:, HM * i:HM * (i + 1)], zinv[:, i:i + 1],
                        cp16[:, C_BLKM:C_BLKM + HM], op0=mult, op1=mult)
                for i in (0, 1):
                    for h in range(H):
                        hs = slice(HM * i + h * M, HM * i + (h + 1) * M)
                        nc.tensor.matmul(tailt[:, 80 + 5 * i:85 + 5 * i],
                                         cp16[:, C_POT:C_POT + 128], utm[:, hs],
                                         start=(h == 0 and i == 0), stop=False,
                                         skip_group_check=True)
                        nc.tensor.matmul(tailt[0:8, 94 + 5 * i:99 + 5 * i],
                                         powl[:], utm[:, hs],
                                         start=False,
                                         stop=(h == H - 1 and i == 1),
                                         skip_group_check=True)
                fqT = work.tile([128, 2 * M], f16, tag="fqT")
                nc.scalar.activation(fqT[:], tailt[:, 80:90],
                                     mybir.ActivationFunctionType.Copy)
                fwl = work.tile([8, 2 * M], f16, tag="fwl")
                nc.vector.tensor_copy(fwl[:], tailt[0:8, 94:104])
                nc.tensor.matmul(tailt[0:MF, 104:114],
                                 cp16[0:8, C_R8:C_R8 + MF], fwl[:],
                                 skip_group_check=True)
                lhsT3 = work.tile([64 + MF, 2 * M], f16, tag="lhsT3")
                for i in (0, 1):
                    nb = 64 * i
                    nc.vector.tensor_tensor(
                        lhsT3[nb:nb + MF, 5 * i:5 * (i + 1)],
                        tailt[0:MF, 104 + 5 * i:109 + 5 * i],
                        cp16[0:MF, C_MASKB5:C_MASKB5 + M],
                        op=mult)
                st0["tailt"] = st1["tailt"] = tailt
                st0["fqT"] = st1["fqT"] = fqT
                st0["lhsT3"] = st1["lhsT3"] = lhsT3

            def stage_c(p):
                """Logits + joint softmax + output for batch pair."""
                b0, b1 = 2 * p, 2 * p + 1
                st0, st1 = state.pop(b0), state.pop(b1)
                tailt = st0["tailt"]
                fqT, lhsT3 = st0["fqT"], st0["lhsT3"]
                for i, st in ((0, st0), (1, st1)):
                    blob, ndft_t, nb = st["blob"], st["ndft"], st["nb"]
                    for c in range(NCHUNK):
                        cs = slice(40 * i + c * M, 40 * i + (c + 1) * M)
                        nc.tensor.matmul(tailt[:, cs],
                                         blob[:, O_LKST + c * 128:O_LKST + (c + 1) * 128],
                                         fqT[:, 5 * i:5 * (i + 1)],
                                         start=True, stop=False,
                                         skip_group_check=True)
                        nc.tensor.matmul(tailt[:, cs],
                                         ndft_t[nb:nb + MF, c * 128:(c + 1) * 128],
                                         lhsT3[nb:nb + MF, 5 * i:5 * (i + 1)],
                                         start=False, stop=True,
                                         skip_group_check=True)
                tl = work.tile([128, 2 * NCHUNK * M], f32, tag="tl")
                nc.scalar.activation(tl[:], tailt[:, 0:80], TANH,
                                     scale=1.0 / math.sqrt(D))
                pl = work.tile([128, 2 * NCHUNK * M], f32, tag="pl")
                for i, st in ((0, st0), (1, st1)):
                    nc.vector.scalar_tensor_tensor(
                        pl[:, 40 * i:40 * (i + 1)], tl[:, 40 * i:40 * (i + 1)],
                        10.0, st["blob"][:, O_MASKT:O_MASKT + 40],
                        op0=mult, op1=add)
                eL = work.tile([128, 2 * NCHUNK * M], f32, tag="eL")
                rL = work.tile([128, 2], f32, tag="rL")
                nc.scalar.activation(eL[:, 0:40], pl[:, 0:40], EXP,
                                     accum_out=rL[:, 0:1])
                nc.scalar.activation(eL[:, 40:80], pl[:, 40:80], EXP,
                                     accum_out=rL[:, 1:2])
                nc.tensor.matmul(tailt[:, 92:94],
                                 cp32[:, F_ONEM:F_ONEM + 128], rL[:],
                                 skip_group_check=True)
                zbinv = work.tile([128, 2], f32, tag="zbinv")
                nc.vector.reciprocal(zbinv[:], tailt[:, 92:94])
                for i, b in ((0, b0), (1, b1)):
                    nc.vector.tensor_scalar_mul(
                        out_all[:, b * NCHUNK * M:(b + 1) * NCHUNK * M],
                        eL[:, 40 * i:40 * (i + 1)], zbinv[:, i:i + 1])

            import os
            STAGES = int(os.environ.get("KV2_STAGES", "3"))
            LAG = int(os.environ.get("KV2_LAG", "1"))
            ORDER = os.environ.get("KV2_ORDER", "abc")
            _stage_map = _CACHE.setdefault("stage_map", [])

            def _mark(tag, fn, *a):
                i0 = nc.next_id()
                fn(*a)
                _stage_map.append((tag, i0, nc.next_id()))

            for t in range(BL + 4):
                for s in ORDER:
                    if s == "a" and t < BL:
                        _mark(f"A{t}", stage_a, t)
                    if (s == "b" and STAGES >= 2 and t % 2 == 1
                            and 0 <= (t - 1) // 2 < BL // 2):
                        _mark(f"B{(t - 1) // 2}", stage_b, (t - 1) // 2)
                    if (s == "c" and STAGES >= 3 and t % 2 == 0
                            and 0 <= (t - 4) // 2 < BL // 2):
                        _mark(f"C{(t - 4) // 2}", stage_c, (t - 4) // 2)
            if STAGES < 3:
                nc.vector.memset(out_all[:], 0.0)
            half = BL * NCHUNK * M // 2
            nc.sync.dma_start(d_out.ap()[:, 0:half], out_all[:, 0:half])
            nc.sync.dma_start(d_out.ap()[:, half:], out_all[:, half:])

    nc.compile()
    return nc


def _prep_inputs(inputs):
    """Host-side shard + relayout (numpy moves/casts only)."""
    f16 = np.float16
    f32 = np.float32
    gks = np.asarray(inputs["glimpse_K_static"], f32)   # [H,B,1,N,KS]
    gvs = np.asarray(inputs["glimpse_V_static"], f32)
    lks = np.asarray(inputs["logit_K_static"], f32)     # [B,1,N,D]
    ndf = np.asarray(inputs["node_dynamic_features"], f32)  # [B,M,N,8]
    mask = np.asarray(inputs["feasibility_mask"])       # [B,M,N] bool
    prev = np.asarray(inputs["prev_node_embeddings"], f32)  # [B,M,D]
    veh = np.asarray(inputs["vehicle_dynamic_features"], f32)  # [B,M,3]
    fc = np.asarray(inputs["fixed_context"], f32)       # [B,1,D]
    W_pcv = np.asarray(inputs["W_pcv"], f32)            # [D, D+3]
    W_pns = np.asarray(inputs["W_pns"], f32)            # [3D, 8]
    po = np.asarray(inputs["po_weight"], f32)           # [D, D]

    blob = np.zeros((B, 128, BLOBC), dtype=f16)
    # kst: rows (h,k), cols n
    blob[:, :, O_KST:O_KST + N] = (
        gks[:, :, 0].transpose(1, 0, 3, 2).reshape(B, 128, N))
    # lkst: rows d, cols n
    blob[:, :, O_LKST:O_LKST + N] = lks[:, 0].transpose(0, 2, 1)
    # vst: [n%128, (c, (h,k))]
    vpad = np.zeros((B, NPAD, 128), dtype=f16)
    vpad[:, :N, :] = gvs[:, :, 0].transpose(1, 2, 0, 3).reshape(B, N, 128)
    blob[:, :, O_VST:O_VST + NCHUNK * 128] = (
        vpad.reshape(B, NCHUNK, 128, 128).transpose(0, 2, 1, 3)
        .reshape(B, 128, NCHUNK * 128))
    # ndfn: [n%128, (c, (m,f)+ones)]; ones only for real n
    npad = np.zeros((B, NPAD, MF1), dtype=f16)
    npad[:, :N, :MF] = ndf.transpose(0, 2, 1, 3).reshape(B, N, MF)
    npad[:, :N, MF] = 1.0
    blob[:, :, O_NDFN:O_NDFN + NCHUNK * MF1] = (
        npad.reshape(B, NCHUNK, 128, MF1).transpose(0, 2, 1, 3)
        .reshape(B, 128, NCHUNK * MF1))
    # maskT: [n%128, (c, m)] 0/-60000 add-form for the logits path
    mpad = np.full((B, NPAD, M), MASKVAL, dtype=f16)
    mpad[:, :N, :] = np.where(mask, 0.0, MASKVAL).transpose(0, 2, 1)
    blob[:, :, O_MASKT:O_MASKT + NCHUNK * M] = (
        mpad.reshape(B, NCHUNK, 128, M).transpose(0, 2, 1, 3)
        .reshape(B, 128, NCHUNK * M))
    # mask01: [n%128, (c, m)] 0/1 multiply-form for the attention path
    m01 = np.zeros((B, NPAD, M), dtype=f16)
    m01[:, :N, :] = mask.transpose(0, 2, 1).astype(f16)
    blob[:, :, O_MASK01:O_MASK01 + NCHUNK * M] = (
        m01.reshape(B, NCHUNK, 128, M).transpose(0, 2, 1, 3)
        .reshape(B, 128, NCHUNK * M))

    # ndftp: rows 0:40 (m,f) features; two batches per slice (partition
    # bases 0 and 64)
    ndft1 = np.zeros((B, 128, NPAD), dtype=f16)
    ndft1[:, :MF, :N] = ndf.transpose(0, 1, 3, 2).reshape(B, MF, N)

    # constants
    cp16 = np.zeros((128, C16), dtype=f16)
    r8 = np.zeros((8, MF), dtype=f16)
    for m in range(M):
        for ff in range(8):
            r8[ff, m * 8 + ff] = 1.0
    cp16[0:8, C_R8:C_R8 + MF] = r8
    mf_m = np.arange(MF) // 8                      # m of each (m,f) row
    hm_m = np.arange(HM) % M                       # m of each (h,m) col
    cp16[0:MF, C_MASKMT:C_MASKMT + HM] = (
        mf_m[:, None] == hm_m[None, :]).astype(f16)
    cp16[0:MF, C_MASKB5:C_MASKB5 + M] = (
        mf_m[:, None] == np.arange(M)[None, :]).astype(f16)
    bhm_m = np.arange(BHM) % M                     # m of each (b,h,m) col
    cp16[0:MF, C_MASKMA:C_MASKMA + BHM] = (
        mf_m[:, None] == bhm_m[None, :]).astype(f16)
    cp16[0:MF, C_WVSTK:C_WVSTK + 128] = np.tile(
        W_pns[0:128].T.reshape(1, 8, 128), (M, 1, 1)).reshape(MF, 128)
    d_h = np.arange(128) // KS                     # h of each (h,k) row
    hm_h = np.arange(HM) // M                      # h of each (h,m) col
    cp16[:, C_BLKM:C_BLKM + HM] = (
        d_h[:, None] == hm_h[None, :]).astype(f16)
    cp16[:, C_POT:C_POT + 128] = po.T
    cp16[:, C_WL8:C_WL8 + 8] = W_pns[256:384]
    cp16[:, C_PO:C_PO + 128] = po
    cp16[:, C_ONES] = 1.0
    cp16[:, C_WK8:C_WK8 + 8] = W_pns[128:256]

    cp32 = np.zeros((128, F32C), dtype=f32)
    cp32[:, F_WPCVA:F_WPCVA + 128] = W_pcv[:, 0:128].T
    cp32[0:3, F_WPCVB:F_WPCVB + 128] = W_pcv[:, 128:131].T
    sel = np.zeros((HM, 128), dtype=f32)
    for h in range(H):
        sel[h * M:(h + 1) * M, h * KS:(h + 1) * KS] = 1.0
    cp32[0:HM, F_SELT:F_SELT + 128] = sel
    cp32[:, F_ONEM:F_ONEM + 128] = 1.0

    in_maps = []
    for cid in range(NCORES):
        sl = slice(cid * BL, (cid + 1) * BL)
        c32 = cp32.copy()
        # fc rows per batch + batch selector, prev/veh transposed
        c32[0:BL, F_FC8:F_FC8 + 128] = fc[sl, 0]
        c32[0:BL, F_BSEL2:F_BSEL2 + BM] = (
            np.arange(BL)[:, None] == (np.arange(BM) // M)[None, :])
        c32[:, F_PREVT:F_PREVT + BM] = (
            prev[sl].transpose(2, 0, 1).reshape(128, BM))
        c32[0:3, F_VEHT:F_VEHT + BM] = (
            veh[sl].transpose(2, 0, 1).reshape(3, BM))
        ndp = np.zeros((BL // 2, 128, NPAD), dtype=f16)
        nd = ndft1[sl]
        ndp[:, 0:MF] = nd[0::2, :MF]
        ndp[:, 64:64 + MF] = nd[1::2, :MF]
        in_maps.append(dict(blob=blob[sl], ndftp=ndp,
                            cp16=cp16.copy(), cp32=c32))
    return in_maps


def kernel(**inputs) -> np.ndarray:
    from concourse import bass_utils

    if "nc" not in _CACHE:
        _CACHE["nc"] = _build_program()
    nc = _CACHE["nc"]
    in_maps = _prep_inputs(inputs)
    res = bass_utils.run_bass_kernel_spmd(nc, in_maps, core_ids=list(range(NCORES)))
    outs = []
    for c in range(NCORES):
        o = res.results[c]["out"]                  # [128, (b, c, m)]
        o = (o.reshape(128, BL, NCHUNK, M).transpose(1, 3, 2, 0)
             .reshape(BL, M, NPAD)[:, :, :N].reshape(BL, M * N))
        outs.append(o)
    return np.concatenate(outs, axis=0).astype(np.float32)
